# revision 28
# baseline (speedup 1.0000x reference)
"""Trainium2 Bass kernel for nn_CogAgentDecoderLayer (8-core SPMD).

Fast path (inputs with sorted permuted positions and 256-aligned expert
boundaries — always true for this model's token layout):
  - Self-attn head-TP (2 heads/core), causal block-skip with a single
    [128,128] triangular mask constant (no [S,S] mask DMA).
  - AllToAll of attention ctx (1MB/rank) replaces ReduceScatter: dense
    projection becomes token-local with full-K contraction, so every
    later phase (cross-attn, MLP) is token-parallel on 256 tokens/core.
  - Cross-attn K/V computed sharded over encoder tokens (E/8 per core),
    AllGathered early, overlapped with self-attention compute. Softmax
    denominator folded into the ctx matmul via a ones-column in V.
  - MLP token-parallel: each core streams its block's expert weights
    (gate_up + down) from HBM under the matmuls; no AllGather, no final
    reduce — each core emits its finished [H, 256] output block.
  - rmsnorm 1/rms factors folded into the QKV / cq PSUM->SBUF copies
    (per-token column scaling commutes with the matmuls and rope).

Generic fallback (any routing/positions): original mask-DMA kernel with
ReduceScatter + AllGather, kept verbatim below.
"""
import os
import numpy as np
from contextlib import ExitStack
from concourse import bacc, tile, mybir
from concourse.bass_utils import run_bass_kernel_spmd

NC_ = 8
S, E, H, NH, HD = 2048, 2048, 2048, 16, 128
CH, CC, CHD = 1024, 1024, 64
I = 5504
NI = I // 128          # 43 down-proj K blocks
IS = I // NC_          # 688 (generic path)
ISP = 768              # padded to 6*128 (generic path)
EPS = 1e-5
ROPE_BASE = 10000.0
F32 = mybir.dt.float32
F32R = mybir.dt.float32r
BF16 = mybir.dt.bfloat16
DVE_F32R = True        # DVE may write fp32r tiles directly
RG = [list(range(NC_))]


def _segs(lo, hi, b0, b1, b2):
    pts = sorted({lo, hi, *[b for b in (b0, b1, b2) if lo < b < hi]})
    out = []
    for s, e in zip(pts, pts[1:]):
        ex = []
        if s < b1:
            ex.append(0)
        if b0 <= s < b2:
            ex.append(1)
        out.append((s, e, ex))
    return out


def _chunks(lo, hi, w):
    out = []
    while lo < hi:
        out.append((lo, min(lo + w, hi)))
        lo += w
    return out


def build_fast(b0, b1, b2):
    nc = bacc.Bacc("TRN2", target_bir_lowering=False, debug=False,
                   num_devices=NC_)
    din = lambda n, sh, dt: nc.dram_tensor(n, sh, dt, kind="ExternalInput")
    hT = din("hT", [H, S], BF16)
    resid = din("resid", [H, 256], F32)
    wqkv0 = din("wqkv0", [H, 768], BF16)
    wqkv1 = din("wqkv1", [H, 768], BF16)
    cos2 = din("cos2", [128, S], BF16)
    sin2 = din("sin2", [128, S], BF16)
    rotT = din("rotT", [128, 128], BF16)
    trimask = din("trimask", [128, 128], BF16)
    onesr = din("onesr", [128, 128], F32R)
    onesb = din("onesb", [128, 128], BF16)
    wdense = din("wdense", [H, H], BF16)
    encTs = din("encTs", [CH, 256], BF16)
    wk = din("wk", [CH, CC], BF16)
    wvv = din("wvv", [CH, CC], BF16)
    wcq = din("wcq", [H, CC], BF16)
    wcd = din("wcd", [CC, H], BF16)
    wgu = din("wgu", [H, 2 * I], BF16)
    wdn = din("wdn", [I, H], BF16)
    y = nc.dram_tensor("y", [H, 256], F32, kind="ExternalOutput")

    SC = 1.0 / float(np.sqrt(HD))
    CSC = 1.0 / float(np.sqrt(CHD))
    EXP = mybir.ActivationFunctionType.Exp
    SQ = mybir.ActivationFunctionType.Square
    SQRT = mybir.ActivationFunctionType.Sqrt
    SILU = mybir.ActivationFunctionType.Silu
    r128 = lambda ap: ap.rearrange("(c p) n -> p c n", p=128)

    with tile.TileContext(nc) as tc, ExitStack() as top:
        const = top.enter_context(tc.tile_pool(name="const", bufs=1))
        ones_sb = const.tile([128, 128], F32R)
        nc.sync.dma_start(ones_sb[:], onesr.ap()[:])
        ones_bf = const.tile([128, 128], BF16)
        nc.sync.dma_start(ones_bf[:], onesb.ap()[:])
        rot_sb = const.tile([128, 128], BF16)
        nc.sync.dma_start(rot_sb[:], rotT.ap()[:])
        tri_sb = const.tile([128, 128], BF16)
        nc.sync.dma_start(tri_sb[:], trimask.ap()[:])
        from concourse.masks import make_identity
        ident = const.tile([128, 128], BF16)
        make_identity(nc, ident[:])
        cos_sb = const.tile([128, S], BF16)
        nc.sync.dma_start(cos_sb[:], cos2.ap()[:])
        sin_sb = const.tile([128, S], BF16)
        nc.sync.dma_start(sin_sb[:], sin2.ap()[:])
        eps_sb = const.tile([128, 1], F32)
        nc.vector.memset(eps_sb[:], EPS)

        dram = top.enter_context(tc.tile_pool(name="dram", bufs=1, space="DRAM"))
        kloc = dram.tile([CC, 256], BF16)
        vloc = dram.tile([256, CC], BF16)
        kall = dram.tile([NC_ * CC, 256], BF16, addr_space="Shared")
        vall = dram.tile([NC_ * 256, CC], BF16, addr_space="Shared")
        a2a_in = dram.tile([H, 256], BF16)
        a2a_out = dram.tile([H, 256], BF16)

        scrp = top.enter_context(tc.tile_pool(name="scr", bufs=2))

        # ===== phase 0: cross K/V for this core's E-shard, then AllGather ====
        with ExitStack() as pKV:
            kvp = pKV.enter_context(tc.tile_pool(name="kvp", bufs=1))
            enc_sb = kvp.tile([128, 8, 256], BF16)
            nc.sync.dma_start(enc_sb[:], r128(encTs.ap()))
            kloc_sb = kvp.tile([128, 8, 256], BF16)
            vloc_sb = kvp.tile([128, 2, CC], BF16)
            kvw = pKV.enter_context(tc.tile_pool(name="kvw", bufs=2))
            kvps = pKV.enter_context(tc.tile_pool(name="kvps", bufs=2,
                                                  space="PSUM"))
            for ccb in range(8):
                wkt = kvw.tile([128, 8, 128], BF16, name="wkt", tag="wkt")
                nc.sync.dma_start(wkt[:],
                                  r128(wk.ap()[:, ccb * 128:ccb * 128 + 128]))
                ps = kvps.tile([128, 256], F32, name="kps", tag="kps")
                for kc in range(8):
                    nc.tensor.matmul(ps[:], wkt[:, kc, :],
                                     enc_sb[:, kc, :],
                                     start=(kc == 0), stop=(kc == 7))
                nc.vector.tensor_copy(kloc_sb[:, ccb, :], ps[:])
            for nb in range(2):
                wvt = kvw.tile([128, 8, 512], BF16, name="wvt", tag="wvt")
                nc.sync.dma_start(wvt[:],
                                  r128(wvv.ap()[:, nb * 512:nb * 512 + 512]))
                for tb in range(2):
                    ps = kvps.tile([128, 512], F32, name="vps", tag="vps")
                    for kc in range(8):
                        nc.tensor.matmul(ps[:], enc_sb[:, kc, tb * 128:tb * 128 + 128],
                                         wvt[:, kc, :],
                                         start=(kc == 0), stop=(kc == 7))
                    nc.vector.tensor_copy(vloc_sb[:, tb, nb * 512:nb * 512 + 512],
                                          ps[:])
            nc.sync.dma_start(r128(kloc[:]), kloc_sb[:])
            nc.sync.dma_start(r128(vloc[:]), vloc_sb[:])
        nc.gpsimd.collective_compute(
            "AllGather", mybir.AluOpType.bypass, replica_groups=RG,
            ins=[kloc.opt()], outs=[kall.opt()])
        nc.gpsimd.collective_compute(
            "AllGather", mybir.AluOpType.bypass, replica_groups=RG,
            ins=[vloc.opt()], outs=[vall.opt()])

        pAB = top.enter_context(ExitStack())
        qkp = pAB.enter_context(tc.tile_pool(name="qkp", bufs=1))
        qkv_sb = qkp.tile([128, 6, S], BF16)      # q0 q1 k0 k1 v0 v1
        v_sb = qkp.tile([128, 16, 256], BF16)     # token-major v
        ctx_sb = qkp.tile([128, 2, S], BF16)

        # ===== phase A: h load + rms factors + QKV(*rinv) + rope + vT =====
        with ExitStack() as pA:
            hp = pA.enter_context(tc.tile_pool(name="hp", bufs=1))
            h_sb = hp.tile([128, 16, S], BF16)
            for t0, t1 in _chunks(0, S, 512):
                nc.sync.dma_start(h_sb[:, :, t0:t1], r128(hT.ap())[:, :, t0:t1])
            rinv_sb = hp.tile([128, S], F32)
            with ExitStack() as pA1:
                nrm = pA1.enter_context(tc.tile_pool(name="nrm", bufs=2))
                nps = pA1.enter_context(tc.tile_pool(name="nps", bufs=2,
                                                     space="PSUM"))
                for t0, t1 in _chunks(0, S, 512):
                    pss = nps.tile([128, 512], F32, name="pss", tag="pss")
                    for kc in range(16):
                        sq = nrm.tile([128, 512], F32R, name="sq", tag="sq")
                        nc.scalar.activation(sq[:], h_sb[:, kc, t0:t1], SQ)
                        nc.tensor.matmul(pss[:], ones_sb[:], sq[:],
                                         start=(kc == 0), stop=(kc == 15))
                    rms = nrm.tile([128, 512], F32, name="rms", tag="rms")
                    nc.scalar.activation(rms[:], pss[:], SQRT,
                                         scale=1.0 / H, bias=eps_sb[:])
                    nc.vector.reciprocal(rinv_sb[:, t0:t1], rms[:])
            with ExitStack() as pA2:
                wp = pA2.enter_context(tc.tile_pool(name="wp", bufs=3))
                mps = pA2.enter_context(tc.tile_pool(name="mps", bufs=2,
                                                     space="PSUM"))
                for slot in range(6):
                    wts = []
                    for ex, wsrc in ((0, wqkv0), (1, wqkv1)):
                        wt = wp.tile([128, 16, 128], BF16,
                                     name=f"wq{ex}{slot}", tag=f"wq{ex}")
                        nc.sync.dma_start(
                            wt[:], r128(wsrc.ap()[:, slot * 128:slot * 128 + 128]))
                        wts.append(wt)
                    for t0, t1 in _chunks(0, S, 512):
                        sg = [x for x in _segs(t0, t1, b0, b1, b2) if x[2]]
                        if not sg:
                            continue
                        need = sorted({x for _, _, ex in sg for x in ex})
                        pss_ = {}
                        for x in need:
                            ps = mps.tile([128, 512], F32, name=f"qps{x}",
                                          tag=f"qps{x}")
                            for kc in range(16):
                                nc.tensor.matmul(ps[:], wts[x][:, kc, :],
                                                 h_sb[:, kc, t0:t1],
                                                 start=(kc == 0), stop=(kc == 15))
                            pss_[x] = ps
                        for s, e, ex in sg:
                            if len(ex) == 1:
                                nc.vector.tensor_mul(qkv_sb[:, slot, s:e],
                                                     pss_[ex[0]][:, s - t0:e - t0],
                                                     rinv_sb[:, s:e])
                            else:
                                tmp = scrp.tile([128, 512], F32,
                                                name="qadd", tag="qadd")
                                nc.vector.tensor_add(tmp[:, :e - s],
                                                     pss_[0][:, s - t0:e - t0],
                                                     pss_[1][:, s - t0:e - t0])
                                nc.vector.tensor_mul(qkv_sb[:, slot, s:e],
                                                     tmp[:, :e - s],
                                                     rinv_sb[:, s:e])
                    if b2 < S:
                        nc.vector.memset(qkv_sb[:, slot, b2:S], 0.0)
                # rope on q,k
                for slot in range(4):
                    for t0, t1 in _chunks(0, S, 512):
                        rp = mps.tile([128, 512], F32, name="rps", tag="qps0")
                        nc.tensor.matmul(rp[:], rot_sb[:],
                                         qkv_sb[:, slot, t0:t1],
                                         start=True, stop=True)
                        c1 = scrp.tile([128, 512], F32, name="ropec", tag="ropec")
                        nc.vector.tensor_mul(c1[:], qkv_sb[:, slot, t0:t1],
                                             cos_sb[:, t0:t1])
                        s1 = scrp.tile([128, 512], F32, name="ropes", tag="ropes")
                        nc.vector.tensor_mul(s1[:], rp[:], sin_sb[:, t0:t1])
                        nc.vector.tensor_add(qkv_sb[:, slot, t0:t1],
                                             c1[:], s1[:])
                # v -> token-major via PE transpose
                for hh in range(2):
                    for tt in range(16):
                        tp = mps.tile([128, 512], BF16, name="tps", tag="qps0")
                        nc.tensor.transpose(
                            tp[:, :128],
                            qkv_sb[:, 4 + hh, tt * 128:tt * 128 + 128],
                            ident[:])
                        nc.vector.tensor_copy(v_sb[:, tt, hh * 128:hh * 128 + 128],
                                              tp[:, :128])
        # ===== phase B: causal self-attention =====
        with ExitStack() as pB:
            ap_ = pB.enter_context(tc.tile_pool(name="ap", bufs=4))
            aps = pB.enter_context(tc.tile_pool(name="aps", bufs=4, space="PSUM"))
            accp = pB.enter_context(tc.tile_pool(name="accp", bufs=1, space="PSUM"))
            for ci, (t0, t1) in enumerate(_chunks(0, S, 512)):
                nkt = 4 * ci + 4
                pss_ = [accp.tile([128, 512], F32, name=f"pssum{h}", tag=f"pssum{h}")
                        for h in range(2)]
                psc_ = [accp.tile([128, 512], F32, name=f"psctx{h}", tag=f"psctx{h}")
                        for h in range(2)]
                for kt in range(nkt):
                    lc = 128 * (kt - 4 * ci) if kt >= 4 * ci else 0
                    for hh in range(2):
                        sc = aps.tile([128, 512], F32, name="sc", tag="sc")
                        nc.tensor.matmul(
                            sc[:, lc:512],
                            qkv_sb[:, 2 + hh, kt * 128:kt * 128 + 128],
                            qkv_sb[:, hh, t0 + lc:t1], start=True, stop=True)
                        if kt >= 4 * ci:
                            nc.vector.tensor_add(sc[:, lc:lc + 128],
                                                 sc[:, lc:lc + 128], tri_sb[:])
                        pr = ap_.tile([128, 512], BF16, name="pr", tag="pr")
                        if lc:
                            nc.vector.memset(pr[:, :lc], 0.0)
                        nc.scalar.activation(pr[:, lc:512], sc[:, lc:512],
                                             EXP, scale=SC)
                        nc.tensor.matmul(pss_[hh][:], ones_bf[:], pr[:],
                                         start=(kt == 0), stop=(kt == nkt - 1))
                        nc.tensor.matmul(
                            psc_[hh][:], v_sb[:, kt, hh * 128:hh * 128 + 128],
                            pr[:], start=(kt == 0), stop=(kt == nkt - 1))
                for hh in range(2):
                    rc = ap_.tile([128, 512], F32, name="rc", tag="rc")
                    nc.vector.reciprocal(rc[:], pss_[hh][:])
                    nc.vector.tensor_mul(ctx_sb[:, hh, t0:t1],
                                         psc_[hh][:], rc[:])
        # ===== A2A: ctx [256 dims, S] -> full ctx [H, 256 tokens] =====
        for j in range(8):
            for hh in range(2):
                nc.sync.dma_start(r128(a2a_in[:])[:, 2 * j + hh, :],
                                  ctx_sb[:, hh, j * 256:j * 256 + 256])
        pAB.close()
        nc.gpsimd.collective_compute(
            "AllToAll", mybir.AluOpType.bypass, replica_groups=RG,
            ins=[a2a_in.opt()], outs=[a2a_out.opt()])

        # ===== phase C/D persistent tiles =====
        pCDF = top.enter_context(ExitStack())
        cdp0 = pCDF.enter_context(tc.tile_pool(name="cdp0", bufs=1))
        h2_sb = cdp0.tile([128, 16, 256], F32)
        h2n_sb = cdp0.tile([128, 16, 256], BF16)
        with ExitStack() as pCD:
            cd1 = pCD.enter_context(tc.tile_pool(name="cd1", bufs=1))
            h1_sb = cd1.tile([128, 16, 256], F32R)
            h1b_sb = cd1.tile([128, 16, 256], BF16)
            cq_sb = cd1.tile([128, 8, 256], BF16)
            cctx_sb = cd1.tile([128, 8, 256], BF16)
            k_sb = cd1.tile([128, 8, E], BF16)
            v2_sb = cd1.tile([128, 16, NH * 65], BF16)
            ctxf_sb = cd1.tile([128, 16, 256], BF16)
            resid_sb = cd1.tile([128, 16, 256], F32)
            rinv1 = cd1.tile([128, 256], F32)
            rinv2 = cd1.tile([128, 256], F32)
            sums_sb = cd1.tile([128, 256], BF16)
            nc.vector.memset(sums_sb[:], 0.0)

            # dense inputs first (critical path), then K/V prefetch behind them
            nc.sync.dma_start(ctxf_sb[:], r128(a2a_out[:]))
            nc.sync.dma_start(resid_sb[:], r128(resid.ap()))
            for r in range(8):
                nc.sync.dma_start(k_sb[:, :, r * 256:r * 256 + 256],
                                  r128(kall[r * CC:(r + 1) * CC, :]))
            for tt in range(16):
                nc.sync.dma_start(
                    v2_sb[:, tt, :].rearrange("p (h d) -> p h d", d=65)[:, :, 64:65],
                    onesb.ap()[:, 0:16].rearrange("p (h d) -> p h d", d=1))
                nc.sync.dma_start(
                    v2_sb[:, tt, :].rearrange("p (h d) -> p h d", d=65)[:, :, 0:64],
                    r128(vall[:])[:, tt, :].rearrange("p (h d) -> p h d", d=64))
            dps2 = pCD.enter_context(tc.tile_pool(name="dps2", bufs=2,
                                                  space="PSUM"))
            # ---- dense (token-local, full K) + residual -> h1 ----
            with ExitStack() as pC1:
                dwp = pC1.enter_context(tc.tile_pool(name="dwp", bufs=4))
                for mt in range(16):
                    dwt = dwp.tile([128, 16, 128], BF16, name="dwt", tag="dwt")
                    nc.sync.dma_start(
                        dwt[:], r128(wdense.ap()[:, mt * 128:mt * 128 + 128]))
                    ps = dps2.tile([128, 256], F32, name="dps", tag="psd")
                    for kc in range(16):
                        nc.tensor.matmul(ps[:], dwt[:, kc, :], ctxf_sb[:, kc, :],
                                         start=(kc == 0), stop=(kc == 15))
                    nc.vector.tensor_add(h1_sb[:, mt, :], ps[:],
                                         resid_sb[:, mt, :])
                    nc.vector.tensor_copy(h1b_sb[:, mt, :],
                                          h1_sb[:, mt, :].bitcast(F32))
            # ---- rmsnorm(h1) -> rinv1 ; cq = (wcq.T @ h1) * rinv1 ----
            pss1 = dps2.tile([128, 256], F32, name="pss1", tag="psd")
            for kc in range(16):
                sq = scrp.tile([128, 256], F32R, name="sqd", tag="sqd")
                nc.scalar.activation(sq[:], h1_sb[:, kc, :].bitcast(F32), SQ)
                nc.tensor.matmul(pss1[:], ones_sb[:], sq[:],
                                 start=(kc == 0), stop=(kc == 15))
            rms1 = scrp.tile([128, 256], F32, name="rmsd", tag="rmsd")
            nc.scalar.activation(rms1[:], pss1[:], SQRT,
                                 scale=1.0 / H, bias=eps_sb[:])
            nc.vector.reciprocal(rinv1[:], rms1[:])
            with ExitStack() as pC2:
                cwp = pC2.enter_context(tc.tile_pool(name="cwp", bufs=3))
                for mt in range(8):
                    wcq_t = cwp.tile([128, 16, 128], BF16, name="wcqt", tag="wcqt")
                    nc.sync.dma_start(
                        wcq_t[:], r128(wcq.ap()[:, mt * 128:mt * 128 + 128]))
                    ps = dps2.tile([128, 256], F32, name="cqp", tag="psd")
                    for kc in range(16):
                        nc.tensor.matmul(ps[:], wcq_t[:, kc, :], h1b_sb[:, kc, :],
                                         start=(kc == 0), stop=(kc == 15))
                    nc.vector.tensor_mul(cq_sb[:, mt, :], ps[:], rinv1[:])
            # ---- cross attention (head pairs share one exp tile) ----
            with ExitStack() as pD3:
                cap = pD3.enter_context(tc.tile_pool(name="cap", bufs=4))
                caps = pD3.enter_context(tc.tile_pool(name="caps", bufs=3,
                                                      space="PSUM"))
                cbcs = pD3.enter_context(tc.tile_pool(name="cbcs", bufs=1,
                                                      space="PSUM"))
                cacc = pD3.enter_context(tc.tile_pool(name="cacc", bufs=1,
                                                      space="PSUM"))
                for h in range(NH):
                    g, i = h // 2, h % 2
                    psc = cacc.tile([65, 256], F32, name=f"cpc{h % 2}",
                                    tag=f"cpc{h % 2}")
                    for kt in range(16):
                        sc = caps.tile([128, 256], F32, name="csc", tag="csc")
                        nc.tensor.matmul(
                            sc[:],
                            k_sb[64 * i:64 * i + 64, g,
                                 kt * 128:kt * 128 + 128],
                            cq_sb[64 * i:64 * i + 64, g, :],
                            start=True, stop=True)
                        pr = cap.tile([128, 256], BF16, name="cpr", tag="cpr")
                        nc.scalar.activation(pr[:], sc[:], EXP, scale=CSC)
                        nc.tensor.matmul(
                            psc[:], v2_sb[:, kt, 65 * h:65 * h + 65],
                            pr[:], start=(kt == 0), stop=(kt == 15))
                    nc.vector.tensor_copy(sums_sb[64:65, :], psc[64:65, :])
                    bc = cbcs.tile([64, 256], F32, name="bc", tag="bc")
                    nc.tensor.matmul(bc[:], ones_bf[:, 0:64], sums_sb[:],
                                     start=True, stop=True)
                    rc = cap.tile([64, 256], F32, name="crc", tag="crc")
                    nc.vector.reciprocal(rc[:], bc[:])
                    nc.vector.tensor_mul(cctx_sb[64 * i:64 * i + 64, g, :],
                                         psc[:64, :], rc[:])
            # ---- cdense + h1 -> h2 ; rmsnorm(h2) -> h2n ----
            with ExitStack() as pD4:
                cdw = pD4.enter_context(tc.tile_pool(name="cdw", bufs=3))
                for mt in range(16):
                    wcd_t = cdw.tile([128, 8, 128], BF16, name="wcdt", tag="wcdt")
                    nc.sync.dma_start(
                        wcd_t[:], r128(wcd.ap()[:, mt * 128:mt * 128 + 128]))
                    ps = dps2.tile([128, 256], F32, name="cdp", tag="psd")
                    for kc in range(8):
                        nc.tensor.matmul(ps[:], wcd_t[:, kc, :], cctx_sb[:, kc, :],
                                         start=(kc == 0), stop=(kc == 7))
                    nc.vector.tensor_add(h2_sb[:, mt, :], ps[:],
                                         h1_sb[:, mt, :].bitcast(F32))
            pss2 = dps2.tile([128, 256], F32, name="pss2", tag="psd")
            for kc in range(16):
                sq = scrp.tile([128, 256], F32R, name="sqd2", tag="sqd")
                nc.scalar.activation(sq[:], h2_sb[:, kc, :], SQ)
                nc.tensor.matmul(pss2[:], ones_sb[:], sq[:],
                                 start=(kc == 0), stop=(kc == 15))
            rms2 = scrp.tile([128, 256], F32, name="rmsd2", tag="rmsd")
            nc.scalar.activation(rms2[:], pss2[:], SQRT,
                                 scale=1.0 / H, bias=eps_sb[:])
            nc.vector.reciprocal(rinv2[:], rms2[:])
            for kc in range(16):
                nc.vector.tensor_mul(h2n_sb[:, kc, :], h2_sb[:, kc, :], rinv2[:])
        # ===== phase F: token-local MLP, streamed expert weights =====
        with ExitStack() as pF:
            fac = pF.enter_context(tc.tile_pool(name="fac", bufs=1))
            act_sb = fac.tile([128, NI, 256], BF16)
            with ExitStack() as pF1:
                gwp = pF1.enter_context(tc.tile_pool(name="gwp", bufs=6))
                fps = pF1.enter_context(tc.tile_pool(name="fps", bufs=2,
                                                     space="PSUM"))
                for ib in range(NI):
                    gwt = gwp.tile([128, 16, 256], BF16, name="gwt", tag="gwt")
                    nc.sync.dma_start(
                        gwt[:], r128(wgu.ap()[:, ib * 256:ib * 256 + 256]))
                    pg = fps.tile([128, 256], F32, name="pg", tag="pg")
                    pu = fps.tile([128, 256], F32, name="pu", tag="pu")
                    for kc in range(16):
                        nc.tensor.matmul(pg[:], gwt[:, kc, 0:128],
                                         h2n_sb[:, kc, :],
                                         start=(kc == 0), stop=(kc == 15))
                        nc.tensor.matmul(pu[:], gwt[:, kc, 128:256],
                                         h2n_sb[:, kc, :],
                                         start=(kc == 0), stop=(kc == 15))
                    gs = scrp.tile([128, 256], F32, name="gs", tag="gs")
                    nc.scalar.activation(gs[:], pg[:], SILU)
                    nc.vector.tensor_mul(act_sb[:, ib, :], gs[:], pu[:])
            with ExitStack() as pF2:
                dnp = pF2.enter_context(tc.tile_pool(name="dnp", bufs=3))
                fpd = pF2.enter_context(tc.tile_pool(name="fpd", bufs=2,
                                                     space="PSUM"))
                fout = pF2.enter_context(tc.tile_pool(name="fout", bufs=4))
                for mt in range(16):
                    dnt = dnp.tile([128, NI, 128], BF16, name="dnt", tag="dnt")
                    nc.sync.dma_start(
                        dnt[:], r128(wdn.ap()[:, mt * 128:mt * 128 + 128]))
                    pd = fpd.tile([128, 256], F32, name="pd", tag="pd")
                    for kc in range(NI):
                        nc.tensor.matmul(pd[:], dnt[:, kc, :], act_sb[:, kc, :],
                                         start=(kc == 0), stop=(kc == NI - 1))
                    yt = fout.tile([128, 256], F32, name="yt", tag="yt")
                    nc.vector.tensor_add(yt[:], pd[:], h2_sb[:, mt, :])
                    nc.sync.dma_start(y.ap()[mt * 128:mt * 128 + 128, :], yt[:])
    nc.compile()
    return nc


def build_generic(b0, b1, b2):
    nc = bacc.Bacc("TRN2", target_bir_lowering=False, debug=False,
                   num_devices=NC_)
    din = lambda n, sh, dt: nc.dram_tensor(n, sh, dt, kind="ExternalInput")
    hT = din("hT", [H, S], BF16)
    wqkv0 = din("wqkv0", [H, 768], BF16)
    wqkv1 = din("wqkv1", [H, 768], BF16)
    wd0 = din("wd0", [256, H], F32R)
    wd1 = din("wd1", [256, H], F32R)
    cos2 = din("cos2", [128, S], BF16)
    sin2 = din("sin2", [128, S], BF16)
    rotT = din("rotT", [128, 128], BF16)
    onesr = din("onesr", [128, 128], F32R)
    onesb = din("onesb", [128, 128], BF16)
    zeros = din("zeros", [128, 512], F32R)
    maskneg = din("maskneg", [S, S], BF16)
    resid = din("resid", [H, 256], F32R)
    encT = din("encT", [CH, E], BF16)
    wk = din("wk", [CH, CC], BF16)
    wvv = din("wvv", [CH, CC], BF16)
    wcq = din("wcq", [H, CC], F32R)
    wcd = din("wcd", [CC, H], F32R)
    wgu0 = din("wgu0", [H, 2 * IS], BF16)
    wgu1 = din("wgu1", [H, 2 * IS], BF16)
    wdn0 = din("wdn0", [ISP, H], BF16)
    wdn1 = din("wdn1", [ISP, H], BF16)
    y = nc.dram_tensor("y", [H, S], F32, kind="ExternalOutput")

    SC = 1.0 / float(np.sqrt(HD))
    CSC = 1.0 / float(np.sqrt(CHD))
    EXP = mybir.ActivationFunctionType.Exp
    SQ = mybir.ActivationFunctionType.Square
    SQRT = mybir.ActivationFunctionType.Sqrt
    SILU = mybir.ActivationFunctionType.Silu
    r128 = lambda ap: ap.rearrange("(c p) n -> p c n", p=128)

    with tile.TileContext(nc) as tc, ExitStack() as top:
        const = top.enter_context(tc.tile_pool(name="const", bufs=1))
        ones_sb = const.tile([128, 128], F32R)
        nc.sync.dma_start(ones_sb[:], onesr.ap()[:])
        ones_bf = const.tile([128, 128], BF16)
        nc.sync.dma_start(ones_bf[:], onesb.ap()[:])
        rot_sb = const.tile([128, 128], BF16)
        nc.sync.dma_start(rot_sb[:], rotT.ap()[:])
        from concourse.masks import make_identity
        ident = const.tile([128, 128], BF16)
        make_identity(nc, ident[:])
        cos_sb = const.tile([128, S], BF16)
        nc.sync.dma_start(cos_sb[:], cos2.ap()[:])
        sin_sb = const.tile([128, S], BF16)
        nc.sync.dma_start(sin_sb[:], sin2.ap()[:])
        zer_sb = const.tile([128, 512], F32R)
        nc.sync.dma_start(zer_sb[:], zeros.ap()[:])
        eps_sb = const.tile([128, 1], F32)
        nc.vector.memset(eps_sb[:], EPS)

        dram = top.enter_context(tc.tile_pool(name="dram", bufs=1, space="DRAM"))
        bounce = dram.tile([NC_ * H, 256], F32)
        rs_out = dram.tile([H, 256], F32)
        h2n_bnc = dram.tile([H, 256], BF16)
        h2n_all = dram.tile([NC_ * H, 256], BF16, addr_space="Shared")
        h2out = nc.dram_tensor("h2out", [H, 256], F32, kind="ExternalOutput")

        scrp = top.enter_context(tc.tile_pool(name="scr", bufs=2))

        def vwrite(op, dst, a, bb):
            if DVE_F32R:
                op(dst, a, bb)
            else:
                scr = scrp.tile([dst.shape[0], dst.shape[-1]], F32,
                                name="vscr", tag="vscr")
                op(scr[:], a, bb)
                nc.scalar.copy(dst, scr[:])

        pABC = top.enter_context(ExitStack())
        qkp = pABC.enter_context(tc.tile_pool(name="qkp", bufs=1))
        qkv_sb = qkp.tile([128, 6, S], BF16)      # q0 q1 k0 k1 v0 v1
        v_sb = qkp.tile([128, 16, 256], BF16)     # token-major v
        ctxp = pABC.enter_context(tc.tile_pool(name="ctxp", bufs=1))
        ctx_sb = ctxp.tile([128, 2, S], F32R)

        # ===== phase A: h load + rmsnorm + QKV + rope + vT =====
        with ExitStack() as pA:
            hp = pA.enter_context(tc.tile_pool(name="hp", bufs=1))
            h_sb = hp.tile([128, 16, S], BF16)
            nc.sync.dma_start(h_sb[:], r128(hT.ap()))
            with ExitStack() as pA1:
                nrm = pA1.enter_context(tc.tile_pool(name="nrm", bufs=2))
                nps = pA1.enter_context(tc.tile_pool(name="nps", bufs=2,
                                                     space="PSUM"))
                for t0, t1 in _chunks(0, S, 512):
                    pss = nps.tile([128, 512], F32, name="pss", tag="pss")
                    for kc in range(16):
                        sq = nrm.tile([128, 512], F32R, name="sq", tag="sq")
                        nc.scalar.activation(sq[:], h_sb[:, kc, t0:t1], SQ)
                        nc.tensor.matmul(pss[:], ones_sb[:], sq[:],
                                         start=(kc == 0), stop=(kc == 15))
                    rms = nrm.tile([128, 512], F32, name="rms", tag="rms")
                    nc.scalar.activation(rms[:], pss[:], SQRT,
                                         scale=1.0 / H, bias=eps_sb[:])
                    rinv = nrm.tile([128, 512], F32, name="rinv", tag="rinv")
                    nc.vector.reciprocal(rinv[:], rms[:])
                    for kc in range(16):
                        nc.vector.tensor_mul(h_sb[:, kc, t0:t1],
                                             h_sb[:, kc, t0:t1], rinv[:])
            with ExitStack() as pA2:
                wp = pA2.enter_context(tc.tile_pool(name="wp", bufs=3))
                mps = pA2.enter_context(tc.tile_pool(name="mps", bufs=2,
                                                     space="PSUM"))
                for slot in range(6):
                    wts = []
                    for ex, wsrc in ((0, wqkv0), (1, wqkv1)):
                        wt = wp.tile([128, 16, 128], BF16,
                                     name=f"wq{ex}{slot}", tag=f"wq{ex}")
                        nc.sync.dma_start(
                            wt[:], r128(wsrc.ap()[:, slot * 128:slot * 128 + 128]))
                        wts.append(wt)
                    for t0, t1 in _chunks(0, S, 512):
                        sg = [x for x in _segs(t0, t1, b0, b1, b2) if x[2]]
                        if not sg:
                            continue
                        need = sorted({x for _, _, ex in sg for x in ex})
                        pss_ = {}
                        for x in need:
                            ps = mps.tile([128, 512], F32, name=f"qps{x}",
                                          tag=f"qps{x}")
                            for kc in range(16):
                                nc.tensor.matmul(ps[:], wts[x][:, kc, :],
                                                 h_sb[:, kc, t0:t1],
                                                 start=(kc == 0), stop=(kc == 15))
                            pss_[x] = ps
                        for s, e, ex in sg:
                            if len(ex) == 1:
                                nc.vector.tensor_copy(qkv_sb[:, slot, s:e],
                                                      pss_[ex[0]][:, s - t0:e - t0])
                            else:
                                nc.vector.tensor_add(qkv_sb[:, slot, s:e],
                                                     pss_[0][:, s - t0:e - t0],
                                                     pss_[1][:, s - t0:e - t0])
                    if b2 < S:
                        nc.vector.memset(qkv_sb[:, slot, b2:S], 0.0)
                # rope on q,k
                for slot in range(4):
                    for t0, t1 in _chunks(0, S, 512):
                        rp = mps.tile([128, 512], F32, name="rps", tag="qps0")
                        nc.tensor.matmul(rp[:], rot_sb[:],
                                         qkv_sb[:, slot, t0:t1],
                                         start=True, stop=True)
                        c1 = scrp.tile([128, 512], F32, name="ropec", tag="ropec")
                        nc.vector.tensor_mul(c1[:], qkv_sb[:, slot, t0:t1],
                                             cos_sb[:, t0:t1])
                        s1 = scrp.tile([128, 512], F32, name="ropes", tag="ropes")
                        nc.vector.tensor_mul(s1[:], rp[:], sin_sb[:, t0:t1])
                        nc.vector.tensor_add(qkv_sb[:, slot, t0:t1],
                                             c1[:], s1[:])
                # v -> token-major via PE transpose
                for hh in range(2):
                    for tt in range(16):
                        tp = mps.tile([128, 512], BF16, name="tps", tag="qps0")
                        nc.tensor.transpose(
                            tp[:, :128],
                            qkv_sb[:, 4 + hh, tt * 128:tt * 128 + 128],
                            ident[:])
                        nc.vector.tensor_copy(v_sb[:, tt, hh * 128:hh * 128 + 128],
                                              tp[:, :128])
        # ===== phase B: self-attention (perm order) =====
        with ExitStack() as pB:
            ap_ = pB.enter_context(tc.tile_pool(name="ap", bufs=3))
            aps = pB.enter_context(tc.tile_pool(name="aps", bufs=2, space="PSUM"))
            accp = pB.enter_context(tc.tile_pool(name="accp", bufs=1, space="PSUM"))
            for t0, t1 in _chunks(0, S, 512):
                pss_ = [accp.tile([128, 512], F32, name=f"pssum{h}", tag=f"pssum{h}")
                        for h in range(2)]
                psc_ = [accp.tile([128, 512], F32, name=f"psctx{h}", tag=f"psctx{h}")
                        for h in range(2)]
                for kt in range(16):
                    mt_ = ap_.tile([128, 512], BF16, name="mt", tag="mt")
                    nc.sync.dma_start(
                        mt_[:], maskneg.ap()[kt * 128:kt * 128 + 128, t0:t1])
                    for hh in range(2):
                        sc = aps.tile([128, 512], F32, name="sc", tag="sc")
                        nc.tensor.matmul(
                            sc[:], qkv_sb[:, 2 + hh, kt * 128:kt * 128 + 128],
                            qkv_sb[:, hh, t0:t1], start=True, stop=True)
                        nc.vector.tensor_add(sc[:], sc[:], mt_[:])
                        pr = ap_.tile([128, 512], BF16, name="pr", tag="pr")
                        nc.scalar.activation(pr[:], sc[:], EXP, scale=SC)
                        nc.tensor.matmul(pss_[hh][:], ones_bf[:], pr[:],
                                         start=(kt == 0), stop=(kt == 15))
                        nc.tensor.matmul(
                            psc_[hh][:], v_sb[:, kt, hh * 128:hh * 128 + 128],
                            pr[:], start=(kt == 0), stop=(kt == 15))
                for hh in range(2):
                    rc = ap_.tile([128, 512], F32, name="rc", tag="rc")
                    nc.vector.reciprocal(rc[:], pss_[hh][:])
                    vwrite(nc.vector.tensor_mul, ctx_sb[:, hh, t0:t1],
                           psc_[hh][:], rc[:])
        # ===== phase C: dense (routed) -> bounce -> RS =====
        with ExitStack() as pC:
            dwp = pC.enter_context(tc.tile_pool(name="dwp", bufs=1))
            dps = pC.enter_context(tc.tile_pool(name="dps", bufs=2, space="PSUM"))
            dop = pC.enter_context(tc.tile_pool(name="dop", bufs=4))
            dwts = []
            for ex, wsrc in ((0, wd0), (1, wd1)):
                dwt = dwp.tile([128, 2, H], F32R, name=f"dw{ex}", tag=f"dw{ex}")
                nc.sync.dma_start(dwt[:], r128(wsrc.ap()))
                dwts.append(dwt)
            for tt in range(8):
                t0, t1 = tt * 256, tt * 256 + 256
                sg = _segs(t0, t1, b0, b1, b2)
                live = [x for x in sg if x[2]]
                for mt in range(16):
                    ot = dop.tile([128, 256], F32, name="dot", tag="dot")
                    if live:
                        need = sorted({x for _, _, ex in live for x in ex})
                        pss_ = {}
                        for x in need:
                            ps = dps.tile([128, 256], F32, name=f"dpst{x}",
                                          tag=f"dpst{x}")
                            for kc in range(2):
                                nc.tensor.matmul(
                                    ps[:],
                                    dwts[x][:, kc, mt * 128:mt * 128 + 128],
                                    ctx_sb[:, kc, t0:t1],
                                    start=(kc == 0), stop=(kc == 1))
                            pss_[x] = ps
                        for s, e, ex in sg:
                            if len(ex) == 2:
                                nc.vector.tensor_add(ot[:, s - t0:e - t0],
                                                     pss_[0][:, s - t0:e - t0],
                                                     pss_[1][:, s - t0:e - t0])
                            elif ex:
                                nc.vector.tensor_copy(ot[:, s - t0:e - t0],
                                                      pss_[ex[0]][:, s - t0:e - t0])
                            else:
                                nc.vector.memset(ot[:, s - t0:e - t0], 0.0)
                    else:
                        nc.vector.memset(ot[:], 0.0)
                    nc.sync.dma_start(
                        bounce[tt * H + mt * 128: tt * H + mt * 128 + 128, :],
                        ot[:])
        pABC.close()
        nc.gpsimd.collective_compute(
            "ReduceScatter", mybir.AluOpType.add,
            replica_groups=RG,
            ins=[bounce.opt()], outs=[rs_out.opt()])

        # ===== phase D: cross attention (token-parallel) =====
        with ExitStack() as pD:
            dp = pD.enter_context(tc.tile_pool(name="dp", bufs=1))
            dps2 = pD.enter_context(tc.tile_pool(name="dps2", bufs=2, space="PSUM"))
            h1_sb = dp.tile([128, 16, 256], F32R)
            cq_sb = dp.tile([128, 8, 256], BF16)
            cctx_sb = dp.tile([128, 8, 256], F32R)
            with ExitStack() as pD1:
                d1 = pD1.enter_context(tc.tile_pool(name="d1", bufs=1))
                rs_sb = d1.tile([128, 16, 256], F32)
                nc.sync.dma_start(rs_sb[:], r128(rs_out[:]))
                re_sb = d1.tile([128, 16, 256], F32R)
                nc.sync.dma_start(re_sb[:], r128(resid.ap()))
                for kc in range(16):
                    vwrite(nc.vector.tensor_add, h1_sb[:, kc, :],
                           rs_sb[:, kc, :], re_sb[:, kc, :].bitcast(F32))
                pss = dps2.tile([128, 256], F32, name="psd", tag="psd")
                for kc in range(16):
                    sq = scrp.tile([128, 256], F32R, name="sqd", tag="sqd")
                    nc.scalar.activation(sq[:], h1_sb[:, kc, :].bitcast(F32), SQ)
                    nc.tensor.matmul(pss[:], ones_sb[:], sq[:],
                                     start=(kc == 0), stop=(kc == 15))
                rms = scrp.tile([128, 256], F32, name="rmsd", tag="rmsd")
                nc.scalar.activation(rms[:], pss[:], SQRT, scale=1.0 / H, bias=eps_sb[:])
                rinv = d1.tile([128, 256], F32)
                nc.vector.reciprocal(rinv[:], rms[:])
                h1n_sb = d1.tile([128, 16, 256], F32R)
                for kc in range(16):
                    vwrite(nc.vector.tensor_mul, h1n_sb[:, kc, :],
                           h1_sb[:, kc, :].bitcast(F32), rinv[:])
                for mt in range(8):
                    wcq_t = d1.tile([128, 16, 128], F32R, name="wcqt", tag="wcqt",
                                    bufs=2)
                    nc.sync.dma_start(
                        wcq_t[:], r128(wcq.ap()[:, mt * 128:mt * 128 + 128]))
                    ps = dps2.tile([128, 256], F32, name="cqp", tag="psd")
                    for kc in range(16):
                        nc.tensor.matmul(ps[:],
                                         wcq_t[:, kc, :],
                                         h1n_sb[:, kc, :],
                                         start=(kc == 0), stop=(kc == 15))
                    nc.vector.tensor_copy(cq_sb[:, mt, :], ps[:])
            with ExitStack() as pD2:
                kp = pD2.enter_context(tc.tile_pool(name="kp", bufs=1))
                k_sb = kp.tile([128, 8, E], BF16)
                v_sb2 = kp.tile([128, 16, CC], BF16)
                with ExitStack() as pD2e:
                    ep = pD2e.enter_context(tc.tile_pool(name="ep", bufs=1))
                    enc_sb = ep.tile([128, 8, E], BF16)
                    nc.sync.dma_start(enc_sb[:], r128(encT.ap()))
                    wk_sb = ep.tile([128, 8, CC], BF16)
                    nc.sync.dma_start(wk_sb[:], r128(wk.ap()))
                    wv_sb = ep.tile([128, 8, CC], BF16)
                    nc.sync.dma_start(wv_sb[:], r128(wvv.ap()))
                    for mt in range(8):
                        for n0, n1 in _chunks(0, E, 512):
                            ps = dps2.tile([128, 512], F32, name="kps", tag="kps")
                            for kc in range(8):
                                nc.tensor.matmul(
                                    ps[:], wk_sb[:, kc, mt * 128:mt * 128 + 128],
                                    enc_sb[:, kc, n0:n1],
                                    start=(kc == 0), stop=(kc == 7))
                            nc.vector.tensor_copy(k_sb[:, mt, n0:n1], ps[:])
                    for tt in range(16):
                        for n0, n1 in _chunks(0, CC, 512):
                            ps = dps2.tile([128, 512], F32, name="vps", tag="kps")
                            for kc in range(8):
                                nc.tensor.matmul(
                                    ps[:], enc_sb[:, kc, tt * 128:tt * 128 + 128],
                                    wv_sb[:, kc, n0:n1],
                                    start=(kc == 0), stop=(kc == 7))
                            nc.vector.tensor_copy(v_sb2[:, tt, n0:n1], ps[:])
                with ExitStack() as pD3:
                    cap = pD3.enter_context(tc.tile_pool(name="cap", bufs=3))
                    caps = pD3.enter_context(tc.tile_pool(name="caps", bufs=2,
                                                          space="PSUM"))
                    cacc = pD3.enter_context(tc.tile_pool(name="cacc", bufs=1,
                                                          space="PSUM"))
                    for h in range(NH):
                        kch, koff = h // 2, 64 * (h % 2)
                        pssum = cacc.tile([128, 256], F32, name="cps", tag="cps")
                        psctx = cacc.tile([64, 256], F32, name="cpc", tag="cpc")
                        for kt in range(16):
                            sc = caps.tile([128, 256], F32, name="csc", tag="csc")
                            nc.tensor.matmul(
                                sc[:],
                                k_sb[koff:koff + 64, kch, kt * 128:kt * 128 + 128],
                                cq_sb[koff:koff + 64, kch, :],
                                start=True, stop=True)
                            pr = cap.tile([128, 256], BF16, name="cpr", tag="cpr")
                            nc.scalar.activation(pr[:], sc[:], EXP, scale=CSC)
                            nc.tensor.matmul(pssum[:], ones_bf[:], pr[:],
                                             start=(kt == 0), stop=(kt == 15))
                            nc.tensor.matmul(psctx[:],
                                             v_sb2[:, kt, 64 * h:64 * h + 64],
                                             pr[:], start=(kt == 0), stop=(kt == 15))
                        rc = cap.tile([64, 256], F32, name="crc", tag="crc")
                        nc.vector.reciprocal(rc[:], pssum[:64, :])
                        vwrite(nc.vector.tensor_mul,
                               cctx_sb[koff:koff + 64, kch, :], psctx[:], rc[:])
            # cdense + residual -> h2, rmsnorm -> h2n -> AG
            with ExitStack() as pD4:
                d4 = pD4.enter_context(tc.tile_pool(name="d4", bufs=1))
                h2_sb = d4.tile([128, 16, 256], F32)
                h2n_sb = d4.tile([128, 16, 256], BF16)
                wcd_sb = d4.tile([128, 8, H], F32R)
                nc.sync.dma_start(wcd_sb[:], r128(wcd.ap()))
                for mt in range(16):
                    ps = dps2.tile([128, 256], F32, name="cdp", tag="psd")
                    for kc in range(8):
                        nc.tensor.matmul(ps[:],
                                         wcd_sb[:, kc, mt * 128:mt * 128 + 128],
                                         cctx_sb[:, kc, :],
                                         start=(kc == 0), stop=(kc == 7))
                    nc.vector.tensor_add(h2_sb[:, mt, :], ps[:],
                                         h1_sb[:, mt, :].bitcast(F32))
                pss2 = dps2.tile([128, 256], F32, name="psd2", tag="psd")
                for kc in range(16):
                    sq = scrp.tile([128, 256], F32R, name="sqd2", tag="sqd")
                    nc.scalar.activation(sq[:], h2_sb[:, kc, :], SQ)
                    nc.tensor.matmul(pss2[:], ones_sb[:], sq[:],
                                     start=(kc == 0), stop=(kc == 15))
                rms2 = scrp.tile([128, 256], F32, name="rmsd2", tag="rmsd")
                nc.scalar.activation(rms2[:], pss2[:], SQRT,
                                     scale=1.0 / H, bias=eps_sb[:])
                rinv2 = d4.tile([128, 256], F32)
                nc.vector.reciprocal(rinv2[:], rms2[:])
                for kc in range(16):
                    nc.vector.tensor_mul(h2n_sb[:, kc, :],
                                         h2_sb[:, kc, :], rinv2[:])
                nc.sync.dma_start(r128(h2n_bnc[:]), h2n_sb[:])
                nc.sync.dma_start(r128(h2out.ap()), h2_sb[:])
            nc.gpsimd.collective_compute(
                "AllGather", mybir.AluOpType.bypass,
                replica_groups=RG,
                ins=[h2n_bnc.opt()], outs=[h2n_all.opt()])
        # ===== phase F: MLP (routed by expert ranges, bf16) =====
        with ExitStack() as pF:
            fp = pF.enter_context(tc.tile_pool(name="fp", bufs=1))
            hn_sb = fp.tile([128, 16, S], BF16)
            for r in range(NC_):
                nc.sync.dma_start(hn_sb[:, :, r * 256:r * 256 + 256],
                                  r128(h2n_all[r * H:(r + 1) * H, :]))
            fw = pF.enter_context(tc.tile_pool(name="fw", bufs=1))
            fps = pF.enter_context(tc.tile_pool(name="fps", bufs=1, space="PSUM"))
            fpd = pF.enter_context(tc.tile_pool(name="fpd", bufs=2, space="PSUM"))
            fac = pF.enter_context(tc.tile_pool(name="fac", bufs=2))
            fout = pF.enter_context(tc.tile_pool(name="fout", bufs=4))
            for ex, (lo, hi) in ((0, (0, b1)), (1, (b1, S))):
                gsrc = (wgu0, wgu1)[ex]
                dsrc = (wdn0, wdn1)[ex]
                dn_t = fw.tile([128, 6, H], BF16, name=f"dn{ex}", tag="dn")
                nc.sync.dma_start(dn_t[:], r128(dsrc.ap()))
                gwts = []
                for pi in range(6):
                    gw = 128 if pi < 5 else 48
                    gwt = fw.tile([128, 16, 256], BF16,
                                  name=f"guw{ex}{pi}", tag=f"guw{pi}")
                    nc.sync.dma_start(
                        gwt[:, :, :2 * gw],
                        r128(gsrc.ap()[:, pi * 256:pi * 256 + 2 * gw]))
                    gwts.append(gwt)
                for a0 in range(0, S, 512):
                    c0, c1 = max(a0, lo), min(a0 + 512, hi)
                    if c0 >= c1:
                        continue
                    t0_, W = a0, 512
                    eo, ew = c0 - a0, c1 - c0
                    act = fac.tile([128, 6, 512], BF16, name="act", tag="act")
                    for pi in range(6):
                        gw = 128 if pi < 5 else 48
                        gwt = gwts[pi]
                        pg = fps.tile([128, 512], F32, name="pg", tag="pg")
                        pu = fps.tile([128, 512], F32, name="pu", tag="pu")
                        for kc in range(16):
                            nc.tensor.matmul(pg[:gw, :W], gwt[:, kc, :gw],
                                             hn_sb[:, kc, t0_:t0_ + 512],
                                             start=(kc == 0), stop=(kc == 15))
                            nc.tensor.matmul(pu[:gw, :W], gwt[:, kc, gw:2 * gw],
                                             hn_sb[:, kc, t0_:t0_ + 512],
                                             start=(kc == 0), stop=(kc == 15))
                        gs = scrp.tile([128, 512], F32, name="gs", tag="gs")
                        nc.scalar.activation(gs[:gw, :W], pg[:gw, :W], SILU)
                        nc.vector.tensor_mul(act[:gw, pi, :W],
                                             gs[:gw, :W], pu[:gw, :W])
                    for mt in range(16):
                        pd = fpd.tile([128, 512], F32, name="pd", tag="pd")
                        for pi in range(6):
                            kw = 128 if pi < 5 else 48
                            nc.tensor.matmul(
                                pd[:, :W],
                                dn_t[:kw, pi, mt * 128:mt * 128 + 128],
                                act[:kw, pi, :W],
                                start=(pi == 0), stop=(pi == 5))
                        ot = fout.tile([128, 512], F32, name="fot", tag="fot")
                        nc.vector.tensor_copy(ot[:, eo:eo + ew], pd[:, eo:eo + ew])
                        nc.sync.dma_start(
                            y.ap()[mt * 128:mt * 128 + 128, c0:c1],
                            ot[:, eo:eo + ew])
    nc.compile()
    return nc


_CACHE = {}


def _route(inputs):
    vm = np.asarray(inputs["vision_token_ids"]).astype(bool)
    lm = np.asarray(inputs["language_token_ids"]).astype(bool)
    g0 = np.where(vm & ~lm)[0]; g1 = np.where(vm & lm)[0]
    g2 = np.where(~vm & lm)[0]; g3 = np.where(~vm & ~lm)[0]
    perm = np.concatenate([g0, g1, g2, g3])
    b0 = len(g0); b1 = b0 + len(g1); b2 = b1 + len(g2)
    return perm, b0, b1, b2


def _rope_tabs(inputs, perm):
    pos = np.asarray(inputs["positions"]).astype(np.float32)
    half = HD // 2
    inv_freq = 1.0 / (ROPE_BASE ** (np.arange(half, dtype=np.float32) / half))
    fr = pos[:, None] * inv_freq[None, :]
    cos2 = np.concatenate([np.cos(fr)] * 2, 1).T[:, perm]
    sin2 = np.concatenate([np.sin(fr)] * 2, 1).T[:, perm]
    rot = np.zeros((HD, HD), np.float32)
    rot[np.arange(half), np.arange(half) + half] = -1.0
    rot[np.arange(half) + half, np.arange(half)] = 1.0
    return cos2, sin2, rot


def _kernel_fast(inputs, perm, b0, b1, b2):
    import ml_dtypes
    f32 = lambda x: np.ascontiguousarray(np.asarray(x, np.float32))
    bf = lambda x: np.ascontiguousarray(np.asarray(x).astype(ml_dtypes.bfloat16))
    cos2, sin2, rot = _rope_tabs(inputs, perm)
    tri = np.where(np.arange(128)[:, None] <= np.arange(128)[None, :],
                   0.0, -30000.0).astype(np.float32)

    wln_in = f32(inputs["w_ln_in"])[:, None]
    wln_pa = f32(inputs["w_ln_post_attn"])[:, None]
    wln_pc = f32(inputs["w_ln_post_cross"])[:, None]
    wqkv = [f32(inputs["w_vis_qkv"]) * wln_in, f32(inputs["w_lang_qkv"]) * wln_in]
    wd = [f32(inputs["w_vis_dense"]), f32(inputs["w_lang_dense"])]
    wgu = [f32(inputs["w_vis_gate_up"]) * wln_pc,
           f32(inputs["w_lang_gate_up"]) * wln_pc]
    wdn = [f32(inputs["w_vis_down"]), f32(inputs["w_lang_down"])]
    wkvf = f32(inputs["w_cross_kv"])
    hTp = f32(inputs["hidden_states"]).T[:, perm].copy()
    encTp = f32(inputs["encoder_embeds"]).T

    def gu_interleave(w):  # w [H, 2I] = [gate | up] -> per-128 [g_i | u_i]
        cols = []
        for i in range(NI):
            cols.append(w[:, 128 * i:128 * i + 128])
            cols.append(w[:, I + 128 * i:I + 128 * i + 128])
        return np.ascontiguousarray(np.concatenate(cols, 1))

    key = ("fast", b0, b1, b2)
    if key not in _CACHE:
        _CACHE.clear()
        _CACHE[key] = build_fast(b0, b1, b2)
    nc = _CACHE[key]

    wdense_cls = {}
    in_maps = []
    for c in range(NC_):
        qs = slice(256 * c, 256 * c + 256)
        t0 = 256 * c
        # dense weight class for this token block
        if t0 < b0:
            dcls = 0            # vis only
        elif t0 < b1:
            dcls = 1            # both
        elif t0 < b2:
            dcls = 2            # lang only
        else:
            dcls = 3            # neither
        if dcls not in wdense_cls:
            wdense_cls[dcls] = (
                wd[0] if dcls == 0 else
                wd[0] + wd[1] if dcls == 1 else
                wd[1] if dcls == 2 else
                np.zeros((H, H), np.float32))
        mex = 0 if t0 < b1 else 1   # MLP expert
        m = dict(
            hT=bf(hTp),
            resid=hTp[:, qs].copy(),
            wqkv0=bf(np.concatenate([wqkv[0][:, qs], wqkv[0][:, H:][:, qs],
                                     wqkv[0][:, 2 * H:][:, qs]], 1)),
            wqkv1=bf(np.concatenate([wqkv[1][:, qs], wqkv[1][:, H:][:, qs],
                                     wqkv[1][:, 2 * H:][:, qs]], 1)),
            cos2=bf(cos2), sin2=bf(sin2), rotT=bf(rot.T),
            trimask=bf(tri),
            onesr=np.ones((128, 128), np.float32),
            onesb=np.ones((128, 128), ml_dtypes.bfloat16),
            wdense=bf(wdense_cls[dcls]),
            encTs=bf(encTp[:, qs]),
            wk=bf(wkvf[:, :CC]), wvv=bf(wkvf[:, CC:]),
            wcq=bf(f32(inputs["w_cross_q"]) * wln_pa),
            wcd=bf(f32(inputs["w_cross_dense"])),
            wgu=bf(gu_interleave(wgu[mex])),
            wdn=bf(wdn[mex]),
        )
        in_maps.append(m)

    trace = bool(int(os.environ.get("KTRACE", "0")))
    res = run_bass_kernel_spmd(nc, in_maps, core_ids=list(range(NC_)),
                               trace=trace)
    kernel.last_exec_ns = res.exec_time_ns
    out = np.empty((S, H), np.float32)
    for c in range(NC_):
        out[perm[256 * c:256 * c + 256], :] = res.results[c]["y"].T
    return out


def _kernel_generic(inputs, perm, b0, b1, b2):
    import ml_dtypes
    f32 = lambda x: np.ascontiguousarray(np.asarray(x, np.float32))
    bf = lambda x: np.ascontiguousarray(np.asarray(x).astype(ml_dtypes.bfloat16))
    cos2, sin2, rot = _rope_tabs(inputs, perm)
    op = np.asarray(inputs["positions"])[perm]
    maskneg = np.where(op[None, :] >= op[:, None], 0.0, -30000.0)

    wln_in = f32(inputs["w_ln_in"])[:, None]
    wln_pa = f32(inputs["w_ln_post_attn"])[:, None]
    wln_pc = f32(inputs["w_ln_post_cross"])[:, None]
    wqkv = [f32(inputs["w_vis_qkv"]) * wln_in, f32(inputs["w_lang_qkv"]) * wln_in]
    wd = [f32(inputs["w_vis_dense"]), f32(inputs["w_lang_dense"])]
    wgu = [f32(inputs["w_vis_gate_up"]) * wln_pc,
           f32(inputs["w_lang_gate_up"]) * wln_pc]
    wdn = [f32(inputs["w_vis_down"]), f32(inputs["w_lang_down"])]
    wkvf = f32(inputs["w_cross_kv"])
    hTp = f32(inputs["hidden_states"]).T[:, perm].copy()

    def interleave(w):  # w [H, 2*IS] = [gate | up]
        cols = []
        for i in range(5):
            cols.append(w[:, 128 * i:128 * i + 128])
            cols.append(w[:, IS + 128 * i:IS + 128 * i + 128])
        cols.append(w[:, 640:IS]); cols.append(w[:, IS + 640:2 * IS])
        return np.ascontiguousarray(np.concatenate(cols, 1))

    key = ("generic", b0, b1, b2)
    if key not in _CACHE:
        _CACHE.clear()
        _CACHE[key] = build_generic(b0, b1, b2)
    nc = _CACHE[key]

    in_maps = []
    for c in range(NC_):
        qs = slice(256 * c, 256 * c + 256)
        m = dict(
            hT=bf(hTp),
            wqkv0=bf(np.concatenate([wqkv[0][:, qs], wqkv[0][:, H:][:, qs],
                                     wqkv[0][:, 2 * H:][:, qs]], 1)),
            wqkv1=bf(np.concatenate([wqkv[1][:, qs], wqkv[1][:, H:][:, qs],
                                     wqkv[1][:, 2 * H:][:, qs]], 1)),
            wd0=wd[0][qs].copy(), wd1=wd[1][qs].copy(),
            cos2=bf(cos2), sin2=bf(sin2), rotT=bf(rot.T),
            onesr=np.ones((128, 128), np.float32),
            onesb=np.ones((128, 128), ml_dtypes.bfloat16),
            zeros=np.zeros((128, 512), np.float32),
            maskneg=bf(maskneg), resid=hTp[:, qs].copy(),
            encT=bf(f32(inputs["encoder_embeds"]).T),
            wk=bf(wkvf[:, :CC]), wvv=bf(wkvf[:, CC:]),
            wcq=(f32(inputs["w_cross_q"]) * wln_pa).copy(),
            wcd=f32(inputs["w_cross_dense"]),
            wgu0=bf(interleave(np.concatenate(
                [wgu[0][:, IS * c:IS * c + IS],
                 wgu[0][:, I + IS * c:I + IS * c + IS]], 1))),
            wgu1=bf(interleave(np.concatenate(
                [wgu[1][:, IS * c:IS * c + IS],
                 wgu[1][:, I + IS * c:I + IS * c + IS]], 1))),
            wdn0=bf(np.concatenate([wdn[0][IS * c:IS * c + IS],
                                    np.zeros((ISP - IS, H), np.float32)], 0)),
            wdn1=bf(np.concatenate([wdn[1][IS * c:IS * c + IS],
                                    np.zeros((ISP - IS, H), np.float32)], 0)),
        )
        in_maps.append(m)

    trace = bool(int(os.environ.get("KTRACE", "0")))
    res = run_bass_kernel_spmd(nc, in_maps, core_ids=list(range(NC_)),
                               trace=trace)
    kernel.last_exec_ns = res.exec_time_ns
    tot = res.results[0]["y"].astype(np.float64)
    for c in range(1, NC_):
        tot += res.results[c]["y"]
    for c in range(NC_):
        tot[:, 256 * c:256 * c + 256] += res.results[c]["h2out"]
    out = np.empty((S, H), np.float32)
    out[perm, :] = tot.T.astype(np.float32)
    return out


def kernel(**inputs):
    perm, b0, b1, b2 = _route(inputs)
    op = np.asarray(inputs["positions"])[perm].astype(np.int64)
    fast = (b0 % 256 == 0 and b1 % 256 == 0 and b2 % 256 == 0
            and op.size == S and np.all(np.diff(op) > 0))
    if fast and not bool(int(os.environ.get("KFORCE_GENERIC", "0"))):
        return _kernel_fast(inputs, perm, b0, b1, b2)
    return _kernel_generic(inputs, perm, b0, b1, b2)


# revision 34
# speedup vs baseline: 1.0554x; 1.0554x over previous
"""Trainium2 Bass kernel for nn_CogAgentDecoderLayer (8-core SPMD).

Fast path (inputs with sorted permuted positions and 256-aligned expert
boundaries — always true for this model's token layout):
  - Self-attn head-TP (2 heads/core), causal block-skip with a single
    [128,128] triangular mask constant (no [S,S] mask DMA).
  - AllToAll of attention ctx (1MB/rank) replaces ReduceScatter: dense
    projection becomes token-local with full-K contraction, so every
    later phase (cross-attn, MLP) is token-parallel on 256 tokens/core.
  - Cross-attn K/V computed sharded over encoder tokens (E/8 per core),
    AllGathered early, overlapped with self-attention compute. Softmax
    denominator folded into the ctx matmul via a ones-column in V.
  - MLP token-parallel: each core streams its block's expert weights
    (gate_up + down) from HBM under the matmuls; no AllGather, no final
    reduce — each core emits its finished [H, 256] output block.
  - rmsnorm 1/rms factors folded into the QKV / cq PSUM->SBUF copies
    (per-token column scaling commutes with the matmuls and rope).

Generic fallback (any routing/positions): original mask-DMA kernel with
ReduceScatter + AllGather, kept verbatim below.
"""
import os
import numpy as np
from contextlib import ExitStack
from concourse import bacc, tile, mybir
from concourse.bass_utils import run_bass_kernel_spmd

NC_ = 8
S, E, H, NH, HD = 2048, 2048, 2048, 16, 128
CH, CC, CHD = 1024, 1024, 64
I = 5504
NI = I // 128          # 43 down-proj K blocks
IS = I // NC_          # 688 (generic path)
ISP = 768              # padded to 6*128 (generic path)
EPS = 1e-5
ROPE_BASE = 10000.0
F32 = mybir.dt.float32
F32R = mybir.dt.float32r
BF16 = mybir.dt.bfloat16
DVE_F32R = True        # DVE may write fp32r tiles directly
RG = [list(range(NC_))]


def _segs(lo, hi, b0, b1, b2):
    pts = sorted({lo, hi, *[b for b in (b0, b1, b2) if lo < b < hi]})
    out = []
    for s, e in zip(pts, pts[1:]):
        ex = []
        if s < b1:
            ex.append(0)
        if b0 <= s < b2:
            ex.append(1)
        out.append((s, e, ex))
    return out


def _chunks(lo, hi, w):
    out = []
    while lo < hi:
        out.append((lo, min(lo + w, hi)))
        lo += w
    return out


def build_fast(b0, b1, b2):
    nc = bacc.Bacc("TRN2", target_bir_lowering=False, debug=False,
                   num_devices=NC_)
    din = lambda n, sh, dt: nc.dram_tensor(n, sh, dt, kind="ExternalInput")
    hT = din("hT", [H, S], BF16)
    resid = din("resid", [H, 256], F32R)
    wqkv0 = din("wqkv0", [H, 768], BF16)
    wqkv1 = din("wqkv1", [H, 768], BF16)
    cos2 = din("cos2", [128, S], BF16)
    sin2 = din("sin2", [128, S], BF16)
    rotT = din("rotT", [128, 128], BF16)
    trimask = din("trimask", [128, 128], BF16)
    onesr = din("onesr", [128, 128], F32R)
    onesb = din("onesb", [128, 128], BF16)
    wdense = din("wdense", [H, H], BF16)
    encTs = din("encTs", [CH, 256], BF16)
    wk = din("wk", [CH, CC], BF16)
    wvv = din("wvv", [CH, CC], BF16)
    wcq = din("wcq", [H, CC], BF16)
    wcd = din("wcd", [CC, H], BF16)
    wgu = din("wgu", [H, 2 * I], BF16)
    wdn = din("wdn", [I, H], BF16)
    y = nc.dram_tensor("y", [H, 256], F32, kind="ExternalOutput")

    SC = 1.0 / float(np.sqrt(HD))
    CSC = 1.0 / float(np.sqrt(CHD))
    EXP = mybir.ActivationFunctionType.Exp
    SQ = mybir.ActivationFunctionType.Square
    SQRT = mybir.ActivationFunctionType.Sqrt
    SILU = mybir.ActivationFunctionType.Silu
    r128 = lambda ap: ap.rearrange("(c p) n -> p c n", p=128)

    with tile.TileContext(nc) as tc, ExitStack() as top:
        const = top.enter_context(tc.tile_pool(name="const", bufs=1))
        ones_sb = const.tile([128, 128], F32R)
        nc.sync.dma_start(ones_sb[:], onesr.ap()[:])
        ones_bf = const.tile([128, 128], BF16)
        nc.sync.dma_start(ones_bf[:], onesb.ap()[:])
        rot_sb = const.tile([128, 128], BF16)
        nc.sync.dma_start(rot_sb[:], rotT.ap()[:])
        tri_sb = const.tile([128, 128], BF16)
        nc.sync.dma_start(tri_sb[:], trimask.ap()[:])
        from concourse.masks import make_identity
        ident = const.tile([128, 128], BF16)
        make_identity(nc, ident[:])
        cos_sb = const.tile([128, S], BF16)
        nc.sync.dma_start(cos_sb[:], cos2.ap()[:])
        sin_sb = const.tile([128, S], BF16)
        nc.sync.dma_start(sin_sb[:], sin2.ap()[:])
        eps_sb = const.tile([128, 1], F32)
        nc.vector.memset(eps_sb[:], EPS)

        dram = top.enter_context(tc.tile_pool(name="dram", bufs=1, space="DRAM"))
        kloc = dram.tile([CC, 256], BF16)
        vloc = dram.tile([256, CC], BF16)
        kall = dram.tile([NC_ * CC, 256], BF16, addr_space="Shared")
        vall = dram.tile([NC_ * 256, CC], BF16, addr_space="Shared")
        a2a_in = dram.tile([H, 256], BF16)
        a2a_out = dram.tile([H, 256], BF16)

        scrp = top.enter_context(tc.tile_pool(name="scr", bufs=2))

        # ===== phase 0: cross K/V for this core's E-shard, then AllGather ====
        with ExitStack() as pKV:
            kvp = pKV.enter_context(tc.tile_pool(name="kvp", bufs=1))
            enc_sb = kvp.tile([128, 8, 256], BF16)
            nc.sync.dma_start(enc_sb[:], r128(encTs.ap()))
            kloc_sb = kvp.tile([128, 8, 256], BF16)
            vloc_sb = kvp.tile([128, 2, CC], BF16)
            kvw = pKV.enter_context(tc.tile_pool(name="kvw", bufs=2))
            kvps = pKV.enter_context(tc.tile_pool(name="kvps", bufs=2,
                                                  space="PSUM"))
            for ccb in range(8):
                wkt = kvw.tile([128, 8, 128], BF16, name="wkt", tag="wkt")
                nc.sync.dma_start(wkt[:],
                                  r128(wk.ap()[:, ccb * 128:ccb * 128 + 128]))
                ps = kvps.tile([128, 256], F32, name="kps", tag="kps")
                for kc in range(8):
                    nc.tensor.matmul(ps[:], wkt[:, kc, :],
                                     enc_sb[:, kc, :],
                                     start=(kc == 0), stop=(kc == 7))
                nc.vector.tensor_copy(kloc_sb[:, ccb, :], ps[:])
            for nb in range(2):
                wvt = kvw.tile([128, 8, 512], BF16, name="wvt", tag="wvt")
                nc.sync.dma_start(wvt[:],
                                  r128(wvv.ap()[:, nb * 512:nb * 512 + 512]))
                for tb in range(2):
                    ps = kvps.tile([128, 512], F32, name="vps", tag="vps")
                    for kc in range(8):
                        nc.tensor.matmul(ps[:], enc_sb[:, kc, tb * 128:tb * 128 + 128],
                                         wvt[:, kc, :],
                                         start=(kc == 0), stop=(kc == 7))
                    nc.vector.tensor_copy(vloc_sb[:, tb, nb * 512:nb * 512 + 512],
                                          ps[:])
            nc.sync.dma_start(r128(kloc[:]), kloc_sb[:])
            nc.sync.dma_start(r128(vloc[:]), vloc_sb[:])
        nc.gpsimd.collective_compute(
            "AllGather", mybir.AluOpType.bypass, replica_groups=RG,
            ins=[kloc.opt()], outs=[kall.opt()])
        nc.gpsimd.collective_compute(
            "AllGather", mybir.AluOpType.bypass, replica_groups=RG,
            ins=[vloc.opt()], outs=[vall.opt()])

        pAB = top.enter_context(ExitStack())
        qkp = pAB.enter_context(tc.tile_pool(name="qkp", bufs=1))
        qkv_sb = qkp.tile([128, 6, S], BF16)      # q0 q1 k0 k1 v0 v1
        v_sb = qkp.tile([128, 16, 256], BF16)     # token-major v
        ctx_sb = qkp.tile([128, 2, S], BF16)

        # ===== phase A: h load + rms factors + QKV(*rinv) + rope + vT =====
        with ExitStack() as pA:
            hp = pA.enter_context(tc.tile_pool(name="hp", bufs=1))
            h_sb = hp.tile([128, 16, S], BF16)
            for t0, t1 in _chunks(0, S, 512):
                nc.sync.dma_start(h_sb[:, :, t0:t1], r128(hT.ap())[:, :, t0:t1])
            rinv_sb = hp.tile([128, S], F32)
            with ExitStack() as pA1:
                nrm = pA1.enter_context(tc.tile_pool(name="nrm", bufs=2))
                nps = pA1.enter_context(tc.tile_pool(name="nps", bufs=2,
                                                     space="PSUM"))
                for t0, t1 in _chunks(0, S, 512):
                    pss = nps.tile([128, 512], F32, name="pss", tag="pss")
                    for kc in range(16):
                        sq = nrm.tile([128, 512], F32R, name="sq", tag="sq")
                        nc.scalar.activation(sq[:], h_sb[:, kc, t0:t1], SQ)
                        nc.tensor.matmul(pss[:], ones_sb[:], sq[:],
                                         start=(kc == 0), stop=(kc == 15))
                    rms = nrm.tile([128, 512], F32, name="rms", tag="rms")
                    nc.scalar.activation(rms[:], pss[:], SQRT,
                                         scale=1.0 / H, bias=eps_sb[:])
                    nc.vector.reciprocal(rinv_sb[:, t0:t1], rms[:])
            with ExitStack() as pA2:
                wp = pA2.enter_context(tc.tile_pool(name="wp", bufs=2))
                rsc = pA2.enter_context(tc.tile_pool(name="rsc", bufs=2))
                mps = pA2.enter_context(tc.tile_pool(name="mps", bufs=2,
                                                     space="PSUM"))
                for slot in range(6):
                    wts = []
                    for ex, wsrc in ((0, wqkv0), (1, wqkv1)):
                        wt = wp.tile([128, 16, 128], BF16,
                                     name=f"wq{ex}{slot}", tag=f"wq{ex}")
                        nc.sync.dma_start(
                            wt[:], r128(wsrc.ap()[:, slot * 128:slot * 128 + 128]))
                        wts.append(wt)
                    for t0, t1 in _chunks(0, S, 512):
                        sg = [x for x in _segs(t0, t1, b0, b1, b2) if x[2]]
                        if not sg:
                            continue
                        need = sorted({x for _, _, ex in sg for x in ex})
                        pss_ = {}
                        for x in need:
                            ps = mps.tile([128, 512], F32, name=f"qps{x}",
                                          tag=f"qps{x}")
                            for kc in range(16):
                                nc.tensor.matmul(ps[:], wts[x][:, kc, :],
                                                 h_sb[:, kc, t0:t1],
                                                 start=(kc == 0), stop=(kc == 15))
                            pss_[x] = ps
                        for s, e, ex in sg:
                            if len(ex) == 1:
                                nc.vector.tensor_mul(qkv_sb[:, slot, s:e],
                                                     pss_[ex[0]][:, s - t0:e - t0],
                                                     rinv_sb[:, s:e])
                            else:
                                tmp = rsc.tile([128, 512], F32,
                                               name="qadd", tag="qadd")
                                nc.vector.tensor_add(tmp[:, :e - s],
                                                     pss_[0][:, s - t0:e - t0],
                                                     pss_[1][:, s - t0:e - t0])
                                nc.vector.tensor_mul(qkv_sb[:, slot, s:e],
                                                     tmp[:, :e - s],
                                                     rinv_sb[:, s:e])
                    if b2 < S:
                        nc.vector.memset(qkv_sb[:, slot, b2:S], 0.0)
                # rope on q,k
                for slot in range(4):
                    for t0, t1 in _chunks(0, S, 512):
                        rp = mps.tile([128, 512], F32, name="rps", tag="qps0")
                        nc.tensor.matmul(rp[:], rot_sb[:],
                                         qkv_sb[:, slot, t0:t1],
                                         start=True, stop=True)
                        c1 = rsc.tile([128, 512], F32, name="ropec", tag="ropec")
                        nc.vector.tensor_mul(c1[:], qkv_sb[:, slot, t0:t1],
                                             cos_sb[:, t0:t1])
                        s1 = rsc.tile([128, 512], F32, name="ropes", tag="ropes")
                        nc.vector.tensor_mul(s1[:], rp[:], sin_sb[:, t0:t1])
                        nc.vector.tensor_add(qkv_sb[:, slot, t0:t1],
                                             c1[:], s1[:])
                # v -> token-major via PE transpose
                for hh in range(2):
                    for tt in range(16):
                        tp = mps.tile([128, 512], BF16, name="tps", tag="qps0")
                        nc.tensor.transpose(
                            tp[:, :128],
                            qkv_sb[:, 4 + hh, tt * 128:tt * 128 + 128],
                            ident[:])
                        nc.vector.tensor_copy(v_sb[:, tt, hh * 128:hh * 128 + 128],
                                              tp[:, :128])
        # ===== phase B: causal self-attention (both heads packed per tile) ====
        with ExitStack() as pB:
            ap_ = pB.enter_context(tc.tile_pool(name="ap", bufs=4))
            aps = pB.enter_context(tc.tile_pool(name="aps", bufs=2, space="PSUM"))
            accp = pB.enter_context(tc.tile_pool(name="accp", bufs=1, space="PSUM"))
            for ci, (t0, t1) in enumerate(_chunks(0, S, 512)):
                nkt = 4 * ci + 4
                # both heads packed column-wise: [:, 0:512]=h0, [:, 512:1024]=h1
                pss2 = accp.tile([128, 1024], F32, name="pssum", tag="pssum")
                psc2 = accp.tile([128, 1024], F32, name="psctx", tag="psctx")
                for kt in range(nkt):
                    lc = 128 * (kt - 4 * ci) if kt >= 4 * ci else 0
                    sc2 = aps.tile([128, 1024], F32, name="sc", tag="sc")
                    pr2 = ap_.tile([128, 1024], BF16, name="pr", tag="pr")
                    for hh in range(2):
                        o = 512 * hh
                        nc.tensor.matmul(
                            sc2[:, o + lc:o + 512],
                            qkv_sb[:, 2 + hh, kt * 128:kt * 128 + 128],
                            qkv_sb[:, hh, t0 + lc:t1], start=True, stop=True)
                        if kt >= 4 * ci:
                            nc.vector.tensor_add(sc2[:, o + lc:o + lc + 128],
                                                 sc2[:, o + lc:o + lc + 128],
                                                 tri_sb[:])
                        if lc:
                            nc.vector.memset(pr2[:, o:o + lc], 0.0)
                    if lc:
                        for hh in range(2):
                            o = 512 * hh
                            nc.scalar.activation(pr2[:, o + lc:o + 512],
                                                 sc2[:, o + lc:o + 512],
                                                 EXP, scale=SC)
                    else:
                        nc.scalar.activation(pr2[:], sc2[:], EXP, scale=SC)
                    for hh in range(2):
                        o = 512 * hh
                        nc.tensor.matmul(pss2[:, o:o + 512], ones_bf[:],
                                         pr2[:, o:o + 512],
                                         start=(kt == 0), stop=(kt == nkt - 1))
                        nc.tensor.matmul(
                            psc2[:, o:o + 512],
                            v_sb[:, kt, hh * 128:hh * 128 + 128],
                            pr2[:, o:o + 512],
                            start=(kt == 0), stop=(kt == nkt - 1))
                for hh in range(2):
                    o = 512 * hh
                    rc = ap_.tile([128, 512], F32, name="rc", tag="rc")
                    nc.vector.reciprocal(rc[:], pss2[:, o:o + 512])
                    nc.vector.tensor_mul(ctx_sb[:, hh, t0:t1],
                                         psc2[:, o:o + 512], rc[:])
        # ===== A2A: ctx [256 dims, S] -> full ctx [H, 256 tokens] =====
        for j in range(8):
            for hh in range(2):
                nc.sync.dma_start(r128(a2a_in[:])[:, 2 * j + hh, :],
                                  ctx_sb[:, hh, j * 256:j * 256 + 256])
        pAB.close()
        nc.gpsimd.collective_compute(
            "AllToAll", mybir.AluOpType.bypass, replica_groups=RG,
            ins=[a2a_in.opt()], outs=[a2a_out.opt()])

        # ===== phase C/D persistent tiles =====
        pCDF = top.enter_context(ExitStack())
        cdp0 = pCDF.enter_context(tc.tile_pool(name="cdp0", bufs=1))
        h2_sb = cdp0.tile([128, 16, 256], F32)
        h2n_sb = cdp0.tile([128, 16, 256], BF16)
        with ExitStack() as pCD:
            cd1 = pCD.enter_context(tc.tile_pool(name="cd1", bufs=1))
            h1_sb = cd1.tile([128, 16, 256], F32R)
            h1b_sb = cd1.tile([128, 16, 256], BF16)
            cq_hm = cd1.tile([128, NH, 256], BF16)   # head-major, parts 0:64
            cctx_sb = cd1.tile([128, 8, 256], BF16)
            v2_sb = cd1.tile([128, 16, NH * 65], BF16)
            rinv1 = cd1.tile([128, 256], F32)
            rinv2 = cd1.tile([128, 256], F32)
            sums_sb = cd1.tile([128, 256], BF16)
            nc.vector.memset(sums_sb[:], 0.0)
            nc.sync.dma_start(h1_sb[:], r128(resid.ap()))

            kgp = pCD.enter_context(tc.tile_pool(name="kgp", bufs=1))

            def load_kg(g):
                # head-major K for heads 8g..8g+7 (parts 0:64), all E tokens
                kg = kgp.tile([128, 8, E], BF16, name="kg", tag="kg")
                for j in range(8):
                    hh = 8 * g + j
                    src = kall[:].rearrange("(r c p) n -> p c r n", p=128, c=8)
                    nc.sync.dma_start(
                        kg[0:64, j, :].rearrange("p (r n) -> p r n", r=8),
                        src[64 * (hh % 2):64 * (hh % 2) + 64, hh // 2, :, :])
                return kg

            dps2 = pCD.enter_context(tc.tile_pool(name="dps2", bufs=2,
                                                  space="PSUM"))
            # ---- dense (token-local, full K) + residual -> h1 ----
            with ExitStack() as pC1:
                c1p = pC1.enter_context(tc.tile_pool(name="c1p", bufs=1))
                ctxf_sb = c1p.tile([128, 16, 256], BF16)
                nc.sync.dma_start(ctxf_sb[:], r128(a2a_out[:]))
                dwp = pC1.enter_context(tc.tile_pool(name="dwp", bufs=3))
                for mt in range(16):
                    dwt = dwp.tile([128, 16, 128], BF16, name="dwt", tag="dwt")
                    nc.sync.dma_start(
                        dwt[:], r128(wdense.ap()[:, mt * 128:mt * 128 + 128]))
                    ps = dps2.tile([128, 256], F32, name="dps", tag="psd")
                    for kc in range(16):
                        nc.tensor.matmul(ps[:], dwt[:, kc, :], ctxf_sb[:, kc, :],
                                         start=(kc == 0), stop=(kc == 15))
                    nc.vector.tensor_add(h1_sb[:, mt, :], ps[:],
                                         h1_sb[:, mt, :].bitcast(F32))
                    nc.vector.tensor_copy(h1b_sb[:, mt, :],
                                          h1_sb[:, mt, :].bitcast(F32))
            # ---- rmsnorm(h1) -> rinv1 ; cq (head-major) ----
            pss1 = dps2.tile([128, 256], F32, name="pss1", tag="psd")
            for kc in range(16):
                sq = scrp.tile([128, 256], F32R, name="sqd", tag="sqd")
                nc.scalar.activation(sq[:], h1_sb[:, kc, :].bitcast(F32), SQ)
                nc.tensor.matmul(pss1[:], ones_sb[:], sq[:],
                                 start=(kc == 0), stop=(kc == 15))
            rms1 = scrp.tile([128, 256], F32, name="rmsd", tag="rmsd")
            nc.scalar.activation(rms1[:], pss1[:], SQRT,
                                 scale=1.0 / H, bias=eps_sb[:])
            nc.vector.reciprocal(rinv1[:], rms1[:])
            # K/V prefetch behind the dense-critical DMAs
            kg0 = load_kg(0)
            for tt in range(16):
                nc.sync.dma_start(
                    v2_sb[:, tt, :].rearrange("p (h d) -> p h d", d=65)[:, :, 64:65],
                    onesb.ap()[:, 0:16].rearrange("p (h d) -> p h d", d=1))
                nc.sync.dma_start(
                    v2_sb[:, tt, :].rearrange("p (h d) -> p h d", d=65)[:, :, 0:64],
                    r128(vall[:])[:, tt, :].rearrange("p (h d) -> p h d", d=64))
            with ExitStack() as pC2:
                cwp = pC2.enter_context(tc.tile_pool(name="cwp", bufs=3))
                for mt in range(8):
                    wcq_t = cwp.tile([128, 16, 128], BF16, name="wcqt", tag="wcqt")
                    nc.sync.dma_start(
                        wcq_t[:], r128(wcq.ap()[:, mt * 128:mt * 128 + 128]))
                    for i in range(2):
                        h = 2 * mt + i
                        ps = dps2.tile([64, 256], F32, name="cqp", tag="psd")
                        for kc in range(16):
                            nc.tensor.matmul(
                                ps[:], wcq_t[:, kc, 64 * i:64 * i + 64],
                                h1b_sb[:, kc, :],
                                start=(kc == 0), stop=(kc == 15))
                        nc.vector.tensor_mul(cq_hm[0:64, h, :], ps[:],
                                             rinv1[0:64, :])
            # ---- cross attention: 4-head quads, one exp per quad ----
            with ExitStack() as pD3:
                cap = pD3.enter_context(tc.tile_pool(name="cap", bufs=4))
                caps = pD3.enter_context(tc.tile_pool(name="caps", bufs=2,
                                                      space="PSUM"))
                cacc = pD3.enter_context(tc.tile_pool(name="cacc", bufs=1,
                                                      space="PSUM"))
                for g in range(2):
                    kg = kg0 if g == 0 else load_kg(1)
                    for q in range(2):
                        psc4 = cacc.tile([65, 1024], F32, name="cpc", tag="cpc")
                        for kt in range(16):
                            sc4 = caps.tile([128, 1024], F32, name="csc",
                                            tag="csc")
                            for j in range(4):
                                jj = 4 * q + j
                                h = 8 * g + jj
                                nc.tensor.matmul(
                                    sc4[:, 256 * j:256 * j + 256],
                                    kg[0:64, jj, kt * 128:kt * 128 + 128],
                                    cq_hm[0:64, h, :],
                                    start=True, stop=True)
                            pr4 = cap.tile([128, 1024], BF16, name="cpr",
                                           tag="cpr")
                            nc.scalar.activation(pr4[:], sc4[:], EXP, scale=CSC)
                            for j in range(4):
                                h = 8 * g + 4 * q + j
                                nc.tensor.matmul(
                                    psc4[:, 256 * j:256 * j + 256],
                                    v2_sb[:, kt, 65 * h:65 * h + 65],
                                    pr4[:, 256 * j:256 * j + 256],
                                    start=(kt == 0 and j % 2 == 0),
                                    stop=(kt == 15 and j % 2 == 1))
                        for j in range(4):
                            h = 8 * g + 4 * q + j
                            nc.vector.tensor_copy(
                                sums_sb[64:65, :],
                                psc4[64:65, 256 * j:256 * j + 256])
                            bc = dps2.tile([64, 256], F32, name="bc", tag="psd")
                            nc.tensor.matmul(bc[:], ones_bf[:, 0:64], sums_sb[:],
                                             start=True, stop=True)
                            rc = cap.tile([64, 256], F32, name="crc", tag="crc")
                            nc.vector.reciprocal(rc[:], bc[:])
                            nc.vector.tensor_mul(
                                cctx_sb[64 * (h % 2):64 * (h % 2) + 64,
                                        h // 2, :],
                                psc4[:64, 256 * j:256 * j + 256], rc[:])
            # ---- cdense + h1 -> h2 ; rmsnorm(h2) -> h2n ----
            with ExitStack() as pD4:
                cdw = pD4.enter_context(tc.tile_pool(name="cdw", bufs=3))
                for mt in range(16):
                    wcd_t = cdw.tile([128, 8, 128], BF16, name="wcdt", tag="wcdt")
                    nc.sync.dma_start(
                        wcd_t[:], r128(wcd.ap()[:, mt * 128:mt * 128 + 128]))
                    ps = dps2.tile([128, 256], F32, name="cdp", tag="psd")
                    for kc in range(8):
                        nc.tensor.matmul(ps[:], wcd_t[:, kc, :], cctx_sb[:, kc, :],
                                         start=(kc == 0), stop=(kc == 7))
                    nc.vector.tensor_add(h2_sb[:, mt, :], ps[:],
                                         h1_sb[:, mt, :].bitcast(F32))
            pss2 = dps2.tile([128, 256], F32, name="pss2", tag="psd")
            for kc in range(16):
                sq = scrp.tile([128, 256], F32R, name="sqd2", tag="sqd")
                nc.scalar.activation(sq[:], h2_sb[:, kc, :], SQ)
                nc.tensor.matmul(pss2[:], ones_sb[:], sq[:],
                                 start=(kc == 0), stop=(kc == 15))
            rms2 = scrp.tile([128, 256], F32, name="rmsd2", tag="rmsd")
            nc.scalar.activation(rms2[:], pss2[:], SQRT,
                                 scale=1.0 / H, bias=eps_sb[:])
            nc.vector.reciprocal(rinv2[:], rms2[:])
            for kc in range(16):
                nc.vector.tensor_mul(h2n_sb[:, kc, :], h2_sb[:, kc, :], rinv2[:])
        # ===== phase F: token-local MLP, streamed expert weights =====
        with ExitStack() as pF:
            fac = pF.enter_context(tc.tile_pool(name="fac", bufs=1))
            act_sb = fac.tile([128, NI, 256], BF16)
            dnp = pF.enter_context(tc.tile_pool(name="dnp", bufs=4))
            with ExitStack() as pF1:
                gwp = pF1.enter_context(tc.tile_pool(name="gwp", bufs=6))
                fps = pF1.enter_context(tc.tile_pool(name="fps", bufs=2,
                                                     space="PSUM"))
                for qb in range(0, NI, 4):    # 4 I-blocks per silu batch
                    nb = min(4, NI - qb)
                    pg4 = fps.tile([128, 1024], F32, name="pg", tag="pg")
                    pu4 = fps.tile([128, 1024], F32, name="pu", tag="pu")
                    for j in range(nb):
                        ib = qb + j
                        gwt = gwp.tile([128, 16, 256], BF16, name="gwt",
                                       tag="gwt")
                        nc.sync.dma_start(
                            gwt[:], r128(wgu.ap()[:, ib * 256:ib * 256 + 256]))
                        o = 256 * j
                        for kc in range(16):
                            nc.tensor.matmul(pg4[:, o:o + 256],
                                             gwt[:, kc, 0:128],
                                             h2n_sb[:, kc, :],
                                             start=(kc == 0), stop=(kc == 15))
                            nc.tensor.matmul(pu4[:, o:o + 256],
                                             gwt[:, kc, 128:256],
                                             h2n_sb[:, kc, :],
                                             start=(kc == 0), stop=(kc == 15))
                    gs4 = scrp.tile([128, 1024], F32, name="gs", tag="gs")
                    nc.scalar.activation(gs4[:, :256 * nb], pg4[:, :256 * nb],
                                         SILU)
                    nc.vector.tensor_mul(
                        act_sb[:, qb:qb + nb, :],
                        gs4[:].rearrange("p (c n) -> p c n", n=256)[:, :nb, :],
                        pu4[:].rearrange("p (c n) -> p c n", n=256)[:, :nb, :])
            with ExitStack() as pF2:
                fpd = pF2.enter_context(tc.tile_pool(name="fpd", bufs=3,
                                                     space="PSUM"))
                fout = pF2.enter_context(tc.tile_pool(name="fout", bufs=4))
                for mt in range(16):
                    dnt = dnp.tile([128, NI, 128], BF16, name="dnt", tag="dnt")
                    nc.sync.dma_start(
                        dnt[:], r128(wdn.ap()[:, mt * 128:mt * 128 + 128]))
                    pd = fpd.tile([128, 256], F32, name="pd", tag="pd")
                    for kc in range(NI):
                        nc.tensor.matmul(pd[:], dnt[:, kc, :], act_sb[:, kc, :],
                                         start=(kc == 0), stop=(kc == NI - 1))
                    yt = fout.tile([128, 256], F32, name="yt", tag="yt")
                    nc.vector.tensor_add(yt[:], pd[:], h2_sb[:, mt, :])
                    nc.sync.dma_start(y.ap()[mt * 128:mt * 128 + 128, :], yt[:])
    nc.compile()
    return nc


def build_generic(b0, b1, b2):
    nc = bacc.Bacc("TRN2", target_bir_lowering=False, debug=False,
                   num_devices=NC_)
    din = lambda n, sh, dt: nc.dram_tensor(n, sh, dt, kind="ExternalInput")
    hT = din("hT", [H, S], BF16)
    wqkv0 = din("wqkv0", [H, 768], BF16)
    wqkv1 = din("wqkv1", [H, 768], BF16)
    wd0 = din("wd0", [256, H], F32R)
    wd1 = din("wd1", [256, H], F32R)
    cos2 = din("cos2", [128, S], BF16)
    sin2 = din("sin2", [128, S], BF16)
    rotT = din("rotT", [128, 128], BF16)
    onesr = din("onesr", [128, 128], F32R)
    onesb = din("onesb", [128, 128], BF16)
    zeros = din("zeros", [128, 512], F32R)
    maskneg = din("maskneg", [S, S], BF16)
    resid = din("resid", [H, 256], F32R)
    encT = din("encT", [CH, E], BF16)
    wk = din("wk", [CH, CC], BF16)
    wvv = din("wvv", [CH, CC], BF16)
    wcq = din("wcq", [H, CC], F32R)
    wcd = din("wcd", [CC, H], F32R)
    wgu0 = din("wgu0", [H, 2 * IS], BF16)
    wgu1 = din("wgu1", [H, 2 * IS], BF16)
    wdn0 = din("wdn0", [ISP, H], BF16)
    wdn1 = din("wdn1", [ISP, H], BF16)
    y = nc.dram_tensor("y", [H, S], F32, kind="ExternalOutput")

    SC = 1.0 / float(np.sqrt(HD))
    CSC = 1.0 / float(np.sqrt(CHD))
    EXP = mybir.ActivationFunctionType.Exp
    SQ = mybir.ActivationFunctionType.Square
    SQRT = mybir.ActivationFunctionType.Sqrt
    SILU = mybir.ActivationFunctionType.Silu
    r128 = lambda ap: ap.rearrange("(c p) n -> p c n", p=128)

    with tile.TileContext(nc) as tc, ExitStack() as top:
        const = top.enter_context(tc.tile_pool(name="const", bufs=1))
        ones_sb = const.tile([128, 128], F32R)
        nc.sync.dma_start(ones_sb[:], onesr.ap()[:])
        ones_bf = const.tile([128, 128], BF16)
        nc.sync.dma_start(ones_bf[:], onesb.ap()[:])
        rot_sb = const.tile([128, 128], BF16)
        nc.sync.dma_start(rot_sb[:], rotT.ap()[:])
        from concourse.masks import make_identity
        ident = const.tile([128, 128], BF16)
        make_identity(nc, ident[:])
        cos_sb = const.tile([128, S], BF16)
        nc.sync.dma_start(cos_sb[:], cos2.ap()[:])
        sin_sb = const.tile([128, S], BF16)
        nc.sync.dma_start(sin_sb[:], sin2.ap()[:])
        zer_sb = const.tile([128, 512], F32R)
        nc.sync.dma_start(zer_sb[:], zeros.ap()[:])
        eps_sb = const.tile([128, 1], F32)
        nc.vector.memset(eps_sb[:], EPS)

        dram = top.enter_context(tc.tile_pool(name="dram", bufs=1, space="DRAM"))
        bounce = dram.tile([NC_ * H, 256], F32)
        rs_out = dram.tile([H, 256], F32)
        h2n_bnc = dram.tile([H, 256], BF16)
        h2n_all = dram.tile([NC_ * H, 256], BF16, addr_space="Shared")
        h2out = nc.dram_tensor("h2out", [H, 256], F32, kind="ExternalOutput")

        scrp = top.enter_context(tc.tile_pool(name="scr", bufs=2))

        def vwrite(op, dst, a, bb):
            if DVE_F32R:
                op(dst, a, bb)
            else:
                scr = scrp.tile([dst.shape[0], dst.shape[-1]], F32,
                                name="vscr", tag="vscr")
                op(scr[:], a, bb)
                nc.scalar.copy(dst, scr[:])

        pABC = top.enter_context(ExitStack())
        qkp = pABC.enter_context(tc.tile_pool(name="qkp", bufs=1))
        qkv_sb = qkp.tile([128, 6, S], BF16)      # q0 q1 k0 k1 v0 v1
        v_sb = qkp.tile([128, 16, 256], BF16)     # token-major v
        ctxp = pABC.enter_context(tc.tile_pool(name="ctxp", bufs=1))
        ctx_sb = ctxp.tile([128, 2, S], F32R)

        # ===== phase A: h load + rmsnorm + QKV + rope + vT =====
        with ExitStack() as pA:
            hp = pA.enter_context(tc.tile_pool(name="hp", bufs=1))
            h_sb = hp.tile([128, 16, S], BF16)
            nc.sync.dma_start(h_sb[:], r128(hT.ap()))
            with ExitStack() as pA1:
                nrm = pA1.enter_context(tc.tile_pool(name="nrm", bufs=2))
                nps = pA1.enter_context(tc.tile_pool(name="nps", bufs=2,
                                                     space="PSUM"))
                for t0, t1 in _chunks(0, S, 512):
                    pss = nps.tile([128, 512], F32, name="pss", tag="pss")
                    for kc in range(16):
                        sq = nrm.tile([128, 512], F32R, name="sq", tag="sq")
                        nc.scalar.activation(sq[:], h_sb[:, kc, t0:t1], SQ)
                        nc.tensor.matmul(pss[:], ones_sb[:], sq[:],
                                         start=(kc == 0), stop=(kc == 15))
                    rms = nrm.tile([128, 512], F32, name="rms", tag="rms")
                    nc.scalar.activation(rms[:], pss[:], SQRT,
                                         scale=1.0 / H, bias=eps_sb[:])
                    rinv = nrm.tile([128, 512], F32, name="rinv", tag="rinv")
                    nc.vector.reciprocal(rinv[:], rms[:])
                    for kc in range(16):
                        nc.vector.tensor_mul(h_sb[:, kc, t0:t1],
                                             h_sb[:, kc, t0:t1], rinv[:])
            with ExitStack() as pA2:
                wp = pA2.enter_context(tc.tile_pool(name="wp", bufs=3))
                mps = pA2.enter_context(tc.tile_pool(name="mps", bufs=2,
                                                     space="PSUM"))
                for slot in range(6):
                    wts = []
                    for ex, wsrc in ((0, wqkv0), (1, wqkv1)):
                        wt = wp.tile([128, 16, 128], BF16,
                                     name=f"wq{ex}{slot}", tag=f"wq{ex}")
                        nc.sync.dma_start(
                            wt[:], r128(wsrc.ap()[:, slot * 128:slot * 128 + 128]))
                        wts.append(wt)
                    for t0, t1 in _chunks(0, S, 512):
                        sg = [x for x in _segs(t0, t1, b0, b1, b2) if x[2]]
                        if not sg:
                            continue
                        need = sorted({x for _, _, ex in sg for x in ex})
                        pss_ = {}
                        for x in need:
                            ps = mps.tile([128, 512], F32, name=f"qps{x}",
                                          tag=f"qps{x}")
                            for kc in range(16):
                                nc.tensor.matmul(ps[:], wts[x][:, kc, :],
                                                 h_sb[:, kc, t0:t1],
                                                 start=(kc == 0), stop=(kc == 15))
                            pss_[x] = ps
                        for s, e, ex in sg:
                            if len(ex) == 1:
                                nc.vector.tensor_copy(qkv_sb[:, slot, s:e],
                                                      pss_[ex[0]][:, s - t0:e - t0])
                            else:
                                nc.vector.tensor_add(qkv_sb[:, slot, s:e],
                                                     pss_[0][:, s - t0:e - t0],
                                                     pss_[1][:, s - t0:e - t0])
                    if b2 < S:
                        nc.vector.memset(qkv_sb[:, slot, b2:S], 0.0)
                # rope on q,k
                for slot in range(4):
                    for t0, t1 in _chunks(0, S, 512):
                        rp = mps.tile([128, 512], F32, name="rps", tag="qps0")
                        nc.tensor.matmul(rp[:], rot_sb[:],
                                         qkv_sb[:, slot, t0:t1],
                                         start=True, stop=True)
                        c1 = scrp.tile([128, 512], F32, name="ropec", tag="ropec")
                        nc.vector.tensor_mul(c1[:], qkv_sb[:, slot, t0:t1],
                                             cos_sb[:, t0:t1])
                        s1 = scrp.tile([128, 512], F32, name="ropes", tag="ropes")
                        nc.vector.tensor_mul(s1[:], rp[:], sin_sb[:, t0:t1])
                        nc.vector.tensor_add(qkv_sb[:, slot, t0:t1],
                                             c1[:], s1[:])
                # v -> token-major via PE transpose
                for hh in range(2):
                    for tt in range(16):
                        tp = mps.tile([128, 512], BF16, name="tps", tag="qps0")
                        nc.tensor.transpose(
                            tp[:, :128],
                            qkv_sb[:, 4 + hh, tt * 128:tt * 128 + 128],
                            ident[:])
                        nc.vector.tensor_copy(v_sb[:, tt, hh * 128:hh * 128 + 128],
                                              tp[:, :128])
        # ===== phase B: self-attention (perm order) =====
        with ExitStack() as pB:
            ap_ = pB.enter_context(tc.tile_pool(name="ap", bufs=3))
            aps = pB.enter_context(tc.tile_pool(name="aps", bufs=2, space="PSUM"))
            accp = pB.enter_context(tc.tile_pool(name="accp", bufs=1, space="PSUM"))
            for t0, t1 in _chunks(0, S, 512):
                pss_ = [accp.tile([128, 512], F32, name=f"pssum{h}", tag=f"pssum{h}")
                        for h in range(2)]
                psc_ = [accp.tile([128, 512], F32, name=f"psctx{h}", tag=f"psctx{h}")
                        for h in range(2)]
                for kt in range(16):
                    mt_ = ap_.tile([128, 512], BF16, name="mt", tag="mt")
                    nc.sync.dma_start(
                        mt_[:], maskneg.ap()[kt * 128:kt * 128 + 128, t0:t1])
                    for hh in range(2):
                        sc = aps.tile([128, 512], F32, name="sc", tag="sc")
                        nc.tensor.matmul(
                            sc[:], qkv_sb[:, 2 + hh, kt * 128:kt * 128 + 128],
                            qkv_sb[:, hh, t0:t1], start=True, stop=True)
                        nc.vector.tensor_add(sc[:], sc[:], mt_[:])
                        pr = ap_.tile([128, 512], BF16, name="pr", tag="pr")
                        nc.scalar.activation(pr[:], sc[:], EXP, scale=SC)
                        nc.tensor.matmul(pss_[hh][:], ones_bf[:], pr[:],
                                         start=(kt == 0), stop=(kt == 15))
                        nc.tensor.matmul(
                            psc_[hh][:], v_sb[:, kt, hh * 128:hh * 128 + 128],
                            pr[:], start=(kt == 0), stop=(kt == 15))
                for hh in range(2):
                    rc = ap_.tile([128, 512], F32, name="rc", tag="rc")
                    nc.vector.reciprocal(rc[:], pss_[hh][:])
                    vwrite(nc.vector.tensor_mul, ctx_sb[:, hh, t0:t1],
                           psc_[hh][:], rc[:])
        # ===== phase C: dense (routed) -> bounce -> RS =====
        with ExitStack() as pC:
            dwp = pC.enter_context(tc.tile_pool(name="dwp", bufs=1))
            dps = pC.enter_context(tc.tile_pool(name="dps", bufs=2, space="PSUM"))
            dop = pC.enter_context(tc.tile_pool(name="dop", bufs=4))
            dwts = []
            for ex, wsrc in ((0, wd0), (1, wd1)):
                dwt = dwp.tile([128, 2, H], F32R, name=f"dw{ex}", tag=f"dw{ex}")
                nc.sync.dma_start(dwt[:], r128(wsrc.ap()))
                dwts.append(dwt)
            for tt in range(8):
                t0, t1 = tt * 256, tt * 256 + 256
                sg = _segs(t0, t1, b0, b1, b2)
                live = [x for x in sg if x[2]]
                for mt in range(16):
                    ot = dop.tile([128, 256], F32, name="dot", tag="dot")
                    if live:
                        need = sorted({x for _, _, ex in live for x in ex})
                        pss_ = {}
                        for x in need:
                            ps = dps.tile([128, 256], F32, name=f"dpst{x}",
                                          tag=f"dpst{x}")
                            for kc in range(2):
                                nc.tensor.matmul(
                                    ps[:],
                                    dwts[x][:, kc, mt * 128:mt * 128 + 128],
                                    ctx_sb[:, kc, t0:t1],
                                    start=(kc == 0), stop=(kc == 1))
                            pss_[x] = ps
                        for s, e, ex in sg:
                            if len(ex) == 2:
                                nc.vector.tensor_add(ot[:, s - t0:e - t0],
                                                     pss_[0][:, s - t0:e - t0],
                                                     pss_[1][:, s - t0:e - t0])
                            elif ex:
                                nc.vector.tensor_copy(ot[:, s - t0:e - t0],
                                                      pss_[ex[0]][:, s - t0:e - t0])
                            else:
                                nc.vector.memset(ot[:, s - t0:e - t0], 0.0)
                    else:
                        nc.vector.memset(ot[:], 0.0)
                    nc.sync.dma_start(
                        bounce[tt * H + mt * 128: tt * H + mt * 128 + 128, :],
                        ot[:])
        pABC.close()
        nc.gpsimd.collective_compute(
            "ReduceScatter", mybir.AluOpType.add,
            replica_groups=RG,
            ins=[bounce.opt()], outs=[rs_out.opt()])

        # ===== phase D: cross attention (token-parallel) =====
        with ExitStack() as pD:
            dp = pD.enter_context(tc.tile_pool(name="dp", bufs=1))
            dps2 = pD.enter_context(tc.tile_pool(name="dps2", bufs=2, space="PSUM"))
            h1_sb = dp.tile([128, 16, 256], F32R)
            cq_sb = dp.tile([128, 8, 256], BF16)
            cctx_sb = dp.tile([128, 8, 256], F32R)
            with ExitStack() as pD1:
                d1 = pD1.enter_context(tc.tile_pool(name="d1", bufs=1))
                rs_sb = d1.tile([128, 16, 256], F32)
                nc.sync.dma_start(rs_sb[:], r128(rs_out[:]))
                re_sb = d1.tile([128, 16, 256], F32R)
                nc.sync.dma_start(re_sb[:], r128(resid.ap()))
                for kc in range(16):
                    vwrite(nc.vector.tensor_add, h1_sb[:, kc, :],
                           rs_sb[:, kc, :], re_sb[:, kc, :].bitcast(F32))
                pss = dps2.tile([128, 256], F32, name="psd", tag="psd")
                for kc in range(16):
                    sq = scrp.tile([128, 256], F32R, name="sqd", tag="sqd")
                    nc.scalar.activation(sq[:], h1_sb[:, kc, :].bitcast(F32), SQ)
                    nc.tensor.matmul(pss[:], ones_sb[:], sq[:],
                                     start=(kc == 0), stop=(kc == 15))
                rms = scrp.tile([128, 256], F32, name="rmsd", tag="rmsd")
                nc.scalar.activation(rms[:], pss[:], SQRT, scale=1.0 / H, bias=eps_sb[:])
                rinv = d1.tile([128, 256], F32)
                nc.vector.reciprocal(rinv[:], rms[:])
                h1n_sb = d1.tile([128, 16, 256], F32R)
                for kc in range(16):
                    vwrite(nc.vector.tensor_mul, h1n_sb[:, kc, :],
                           h1_sb[:, kc, :].bitcast(F32), rinv[:])
                for mt in range(8):
                    wcq_t = d1.tile([128, 16, 128], F32R, name="wcqt", tag="wcqt",
                                    bufs=2)
                    nc.sync.dma_start(
                        wcq_t[:], r128(wcq.ap()[:, mt * 128:mt * 128 + 128]))
                    ps = dps2.tile([128, 256], F32, name="cqp", tag="psd")
                    for kc in range(16):
                        nc.tensor.matmul(ps[:],
                                         wcq_t[:, kc, :],
                                         h1n_sb[:, kc, :],
                                         start=(kc == 0), stop=(kc == 15))
                    nc.vector.tensor_copy(cq_sb[:, mt, :], ps[:])
            with ExitStack() as pD2:
                kp = pD2.enter_context(tc.tile_pool(name="kp", bufs=1))
                k_sb = kp.tile([128, 8, E], BF16)
                v_sb2 = kp.tile([128, 16, CC], BF16)
                with ExitStack() as pD2e:
                    ep = pD2e.enter_context(tc.tile_pool(name="ep", bufs=1))
                    enc_sb = ep.tile([128, 8, E], BF16)
                    nc.sync.dma_start(enc_sb[:], r128(encT.ap()))
                    wk_sb = ep.tile([128, 8, CC], BF16)
                    nc.sync.dma_start(wk_sb[:], r128(wk.ap()))
                    wv_sb = ep.tile([128, 8, CC], BF16)
                    nc.sync.dma_start(wv_sb[:], r128(wvv.ap()))
                    for mt in range(8):
                        for n0, n1 in _chunks(0, E, 512):
                            ps = dps2.tile([128, 512], F32, name="kps", tag="kps")
                            for kc in range(8):
                                nc.tensor.matmul(
                                    ps[:], wk_sb[:, kc, mt * 128:mt * 128 + 128],
                                    enc_sb[:, kc, n0:n1],
                                    start=(kc == 0), stop=(kc == 7))
                            nc.vector.tensor_copy(k_sb[:, mt, n0:n1], ps[:])
                    for tt in range(16):
                        for n0, n1 in _chunks(0, CC, 512):
                            ps = dps2.tile([128, 512], F32, name="vps", tag="kps")
                            for kc in range(8):
                                nc.tensor.matmul(
                                    ps[:], enc_sb[:, kc, tt * 128:tt * 128 + 128],
                                    wv_sb[:, kc, n0:n1],
                                    start=(kc == 0), stop=(kc == 7))
                            nc.vector.tensor_copy(v_sb2[:, tt, n0:n1], ps[:])
                with ExitStack() as pD3:
                    cap = pD3.enter_context(tc.tile_pool(name="cap", bufs=3))
                    caps = pD3.enter_context(tc.tile_pool(name="caps", bufs=2,
                                                          space="PSUM"))
                    cacc = pD3.enter_context(tc.tile_pool(name="cacc", bufs=1,
                                                          space="PSUM"))
                    for h in range(NH):
                        kch, koff = h // 2, 64 * (h % 2)
                        pssum = cacc.tile([128, 256], F32, name="cps", tag="cps")
                        psctx = cacc.tile([64, 256], F32, name="cpc", tag="cpc")
                        for kt in range(16):
                            sc = caps.tile([128, 256], F32, name="csc", tag="csc")
                            nc.tensor.matmul(
                                sc[:],
                                k_sb[koff:koff + 64, kch, kt * 128:kt * 128 + 128],
                                cq_sb[koff:koff + 64, kch, :],
                                start=True, stop=True)
                            pr = cap.tile([128, 256], BF16, name="cpr", tag="cpr")
                            nc.scalar.activation(pr[:], sc[:], EXP, scale=CSC)
                            nc.tensor.matmul(pssum[:], ones_bf[:], pr[:],
                                             start=(kt == 0), stop=(kt == 15))
                            nc.tensor.matmul(psctx[:],
                                             v_sb2[:, kt, 64 * h:64 * h + 64],
                                             pr[:], start=(kt == 0), stop=(kt == 15))
                        rc = cap.tile([64, 256], F32, name="crc", tag="crc")
                        nc.vector.reciprocal(rc[:], pssum[:64, :])
                        vwrite(nc.vector.tensor_mul,
                               cctx_sb[koff:koff + 64, kch, :], psctx[:], rc[:])
            # cdense + residual -> h2, rmsnorm -> h2n -> AG
            with ExitStack() as pD4:
                d4 = pD4.enter_context(tc.tile_pool(name="d4", bufs=1))
                h2_sb = d4.tile([128, 16, 256], F32)
                h2n_sb = d4.tile([128, 16, 256], BF16)
                wcd_sb = d4.tile([128, 8, H], F32R)
                nc.sync.dma_start(wcd_sb[:], r128(wcd.ap()))
                for mt in range(16):
                    ps = dps2.tile([128, 256], F32, name="cdp", tag="psd")
                    for kc in range(8):
                        nc.tensor.matmul(ps[:],
                                         wcd_sb[:, kc, mt * 128:mt * 128 + 128],
                                         cctx_sb[:, kc, :],
                                         start=(kc == 0), stop=(kc == 7))
                    nc.vector.tensor_add(h2_sb[:, mt, :], ps[:],
                                         h1_sb[:, mt, :].bitcast(F32))
                pss2 = dps2.tile([128, 256], F32, name="psd2", tag="psd")
                for kc in range(16):
                    sq = scrp.tile([128, 256], F32R, name="sqd2", tag="sqd")
                    nc.scalar.activation(sq[:], h2_sb[:, kc, :], SQ)
                    nc.tensor.matmul(pss2[:], ones_sb[:], sq[:],
                                     start=(kc == 0), stop=(kc == 15))
                rms2 = scrp.tile([128, 256], F32, name="rmsd2", tag="rmsd")
                nc.scalar.activation(rms2[:], pss2[:], SQRT,
                                     scale=1.0 / H, bias=eps_sb[:])
                rinv2 = d4.tile([128, 256], F32)
                nc.vector.reciprocal(rinv2[:], rms2[:])
                for kc in range(16):
                    nc.vector.tensor_mul(h2n_sb[:, kc, :],
                                         h2_sb[:, kc, :], rinv2[:])
                nc.sync.dma_start(r128(h2n_bnc[:]), h2n_sb[:])
                nc.sync.dma_start(r128(h2out.ap()), h2_sb[:])
            nc.gpsimd.collective_compute(
                "AllGather", mybir.AluOpType.bypass,
                replica_groups=RG,
                ins=[h2n_bnc.opt()], outs=[h2n_all.opt()])
        # ===== phase F: MLP (routed by expert ranges, bf16) =====
        with ExitStack() as pF:
            fp = pF.enter_context(tc.tile_pool(name="fp", bufs=1))
            hn_sb = fp.tile([128, 16, S], BF16)
            for r in range(NC_):
                nc.sync.dma_start(hn_sb[:, :, r * 256:r * 256 + 256],
                                  r128(h2n_all[r * H:(r + 1) * H, :]))
            fw = pF.enter_context(tc.tile_pool(name="fw", bufs=1))
            fps = pF.enter_context(tc.tile_pool(name="fps", bufs=1, space="PSUM"))
            fpd = pF.enter_context(tc.tile_pool(name="fpd", bufs=2, space="PSUM"))
            fac = pF.enter_context(tc.tile_pool(name="fac", bufs=2))
            fout = pF.enter_context(tc.tile_pool(name="fout", bufs=4))
            for ex, (lo, hi) in ((0, (0, b1)), (1, (b1, S))):
                gsrc = (wgu0, wgu1)[ex]
                dsrc = (wdn0, wdn1)[ex]
                dn_t = fw.tile([128, 6, H], BF16, name=f"dn{ex}", tag="dn")
                nc.sync.dma_start(dn_t[:], r128(dsrc.ap()))
                gwts = []
                for pi in range(6):
                    gw = 128 if pi < 5 else 48
                    gwt = fw.tile([128, 16, 256], BF16,
                                  name=f"guw{ex}{pi}", tag=f"guw{pi}")
                    nc.sync.dma_start(
                        gwt[:, :, :2 * gw],
                        r128(gsrc.ap()[:, pi * 256:pi * 256 + 2 * gw]))
                    gwts.append(gwt)
                for a0 in range(0, S, 512):
                    c0, c1 = max(a0, lo), min(a0 + 512, hi)
                    if c0 >= c1:
                        continue
                    t0_, W = a0, 512
                    eo, ew = c0 - a0, c1 - c0
                    act = fac.tile([128, 6, 512], BF16, name="act", tag="act")
                    for pi in range(6):
                        gw = 128 if pi < 5 else 48
                        gwt = gwts[pi]
                        pg = fps.tile([128, 512], F32, name="pg", tag="pg")
                        pu = fps.tile([128, 512], F32, name="pu", tag="pu")
                        for kc in range(16):
                            nc.tensor.matmul(pg[:gw, :W], gwt[:, kc, :gw],
                                             hn_sb[:, kc, t0_:t0_ + 512],
                                             start=(kc == 0), stop=(kc == 15))
                            nc.tensor.matmul(pu[:gw, :W], gwt[:, kc, gw:2 * gw],
                                             hn_sb[:, kc, t0_:t0_ + 512],
                                             start=(kc == 0), stop=(kc == 15))
                        gs = scrp.tile([128, 512], F32, name="gs", tag="gs")
                        nc.scalar.activation(gs[:gw, :W], pg[:gw, :W], SILU)
                        nc.vector.tensor_mul(act[:gw, pi, :W],
                                             gs[:gw, :W], pu[:gw, :W])
                    for mt in range(16):
                        pd = fpd.tile([128, 512], F32, name="pd", tag="pd")
                        for pi in range(6):
                            kw = 128 if pi < 5 else 48
                            nc.tensor.matmul(
                                pd[:, :W],
                                dn_t[:kw, pi, mt * 128:mt * 128 + 128],
                                act[:kw, pi, :W],
                                start=(pi == 0), stop=(pi == 5))
                        ot = fout.tile([128, 512], F32, name="fot", tag="fot")
                        nc.vector.tensor_copy(ot[:, eo:eo + ew], pd[:, eo:eo + ew])
                        nc.sync.dma_start(
                            y.ap()[mt * 128:mt * 128 + 128, c0:c1],
                            ot[:, eo:eo + ew])
    nc.compile()
    return nc


_CACHE = {}


def _route(inputs):
    vm = np.asarray(inputs["vision_token_ids"]).astype(bool)
    lm = np.asarray(inputs["language_token_ids"]).astype(bool)
    g0 = np.where(vm & ~lm)[0]; g1 = np.where(vm & lm)[0]
    g2 = np.where(~vm & lm)[0]; g3 = np.where(~vm & ~lm)[0]
    perm = np.concatenate([g0, g1, g2, g3])
    b0 = len(g0); b1 = b0 + len(g1); b2 = b1 + len(g2)
    return perm, b0, b1, b2


def _rope_tabs(inputs, perm):
    pos = np.asarray(inputs["positions"]).astype(np.float32)
    half = HD // 2
    inv_freq = 1.0 / (ROPE_BASE ** (np.arange(half, dtype=np.float32) / half))
    fr = pos[:, None] * inv_freq[None, :]
    cos2 = np.concatenate([np.cos(fr)] * 2, 1).T[:, perm]
    sin2 = np.concatenate([np.sin(fr)] * 2, 1).T[:, perm]
    rot = np.zeros((HD, HD), np.float32)
    rot[np.arange(half), np.arange(half) + half] = -1.0
    rot[np.arange(half) + half, np.arange(half)] = 1.0
    return cos2, sin2, rot


def _kernel_fast(inputs, perm, b0, b1, b2):
    import ml_dtypes
    f32 = lambda x: np.ascontiguousarray(np.asarray(x, np.float32))
    bf = lambda x: np.ascontiguousarray(np.asarray(x).astype(ml_dtypes.bfloat16))
    cos2, sin2, rot = _rope_tabs(inputs, perm)
    tri = np.where(np.arange(128)[:, None] <= np.arange(128)[None, :],
                   0.0, -30000.0).astype(np.float32)

    wln_in = f32(inputs["w_ln_in"])[:, None]
    wln_pa = f32(inputs["w_ln_post_attn"])[:, None]
    wln_pc = f32(inputs["w_ln_post_cross"])[:, None]
    wqkv = [f32(inputs["w_vis_qkv"]) * wln_in, f32(inputs["w_lang_qkv"]) * wln_in]
    wd = [f32(inputs["w_vis_dense"]), f32(inputs["w_lang_dense"])]
    wgu = [f32(inputs["w_vis_gate_up"]) * wln_pc,
           f32(inputs["w_lang_gate_up"]) * wln_pc]
    wdn = [f32(inputs["w_vis_down"]), f32(inputs["w_lang_down"])]
    wkvf = f32(inputs["w_cross_kv"])
    hTp = f32(inputs["hidden_states"]).T[:, perm].copy()
    encTp = f32(inputs["encoder_embeds"]).T

    def gu_interleave(w):  # w [H, 2I] = [gate | up] -> per-128 [g_i | u_i]
        cols = []
        for i in range(NI):
            cols.append(w[:, 128 * i:128 * i + 128])
            cols.append(w[:, I + 128 * i:I + 128 * i + 128])
        return np.ascontiguousarray(np.concatenate(cols, 1))

    key = ("fast", b0, b1, b2)
    if key not in _CACHE:
        _CACHE.clear()
        _CACHE[key] = build_fast(b0, b1, b2)
    nc = _CACHE[key]

    wdense_cls = {}
    in_maps = []
    for c in range(NC_):
        qs = slice(256 * c, 256 * c + 256)
        t0 = 256 * c
        # dense weight class for this token block
        if t0 < b0:
            dcls = 0            # vis only
        elif t0 < b1:
            dcls = 1            # both
        elif t0 < b2:
            dcls = 2            # lang only
        else:
            dcls = 3            # neither
        if dcls not in wdense_cls:
            wdense_cls[dcls] = (
                wd[0] if dcls == 0 else
                wd[0] + wd[1] if dcls == 1 else
                wd[1] if dcls == 2 else
                np.zeros((H, H), np.float32))
        mex = 0 if t0 < b1 else 1   # MLP expert
        m = dict(
            hT=bf(hTp),
            resid=hTp[:, qs].copy(),
            wqkv0=bf(np.concatenate([wqkv[0][:, qs], wqkv[0][:, H:][:, qs],
                                     wqkv[0][:, 2 * H:][:, qs]], 1)),
            wqkv1=bf(np.concatenate([wqkv[1][:, qs], wqkv[1][:, H:][:, qs],
                                     wqkv[1][:, 2 * H:][:, qs]], 1)),
            cos2=bf(cos2), sin2=bf(sin2), rotT=bf(rot.T),
            trimask=bf(tri),
            onesr=np.ones((128, 128), np.float32),
            onesb=np.ones((128, 128), ml_dtypes.bfloat16),
            wdense=bf(wdense_cls[dcls]),
            encTs=bf(encTp[:, qs]),
            wk=bf(wkvf[:, :CC]), wvv=bf(wkvf[:, CC:]),
            wcq=bf(f32(inputs["w_cross_q"]) * wln_pa),
            wcd=bf(f32(inputs["w_cross_dense"])),
            wgu=bf(gu_interleave(wgu[mex])),
            wdn=bf(wdn[mex]),
        )
        in_maps.append(m)

    trace = bool(int(os.environ.get("KTRACE", "0")))
    res = run_bass_kernel_spmd(nc, in_maps, core_ids=list(range(NC_)),
                               trace=trace)
    kernel.last_exec_ns = res.exec_time_ns
    out = np.empty((S, H), np.float32)
    for c in range(NC_):
        out[perm[256 * c:256 * c + 256], :] = res.results[c]["y"].T
    return out


def _kernel_generic(inputs, perm, b0, b1, b2):
    import ml_dtypes
    f32 = lambda x: np.ascontiguousarray(np.asarray(x, np.float32))
    bf = lambda x: np.ascontiguousarray(np.asarray(x).astype(ml_dtypes.bfloat16))
    cos2, sin2, rot = _rope_tabs(inputs, perm)
    op = np.asarray(inputs["positions"])[perm]
    maskneg = np.where(op[None, :] >= op[:, None], 0.0, -30000.0)

    wln_in = f32(inputs["w_ln_in"])[:, None]
    wln_pa = f32(inputs["w_ln_post_attn"])[:, None]
    wln_pc = f32(inputs["w_ln_post_cross"])[:, None]
    wqkv = [f32(inputs["w_vis_qkv"]) * wln_in, f32(inputs["w_lang_qkv"]) * wln_in]
    wd = [f32(inputs["w_vis_dense"]), f32(inputs["w_lang_dense"])]
    wgu = [f32(inputs["w_vis_gate_up"]) * wln_pc,
           f32(inputs["w_lang_gate_up"]) * wln_pc]
    wdn = [f32(inputs["w_vis_down"]), f32(inputs["w_lang_down"])]
    wkvf = f32(inputs["w_cross_kv"])
    hTp = f32(inputs["hidden_states"]).T[:, perm].copy()

    def interleave(w):  # w [H, 2*IS] = [gate | up]
        cols = []
        for i in range(5):
            cols.append(w[:, 128 * i:128 * i + 128])
            cols.append(w[:, IS + 128 * i:IS + 128 * i + 128])
        cols.append(w[:, 640:IS]); cols.append(w[:, IS + 640:2 * IS])
        return np.ascontiguousarray(np.concatenate(cols, 1))

    key = ("generic", b0, b1, b2)
    if key not in _CACHE:
        _CACHE.clear()
        _CACHE[key] = build_generic(b0, b1, b2)
    nc = _CACHE[key]

    in_maps = []
    for c in range(NC_):
        qs = slice(256 * c, 256 * c + 256)
        m = dict(
            hT=bf(hTp),
            wqkv0=bf(np.concatenate([wqkv[0][:, qs], wqkv[0][:, H:][:, qs],
                                     wqkv[0][:, 2 * H:][:, qs]], 1)),
            wqkv1=bf(np.concatenate([wqkv[1][:, qs], wqkv[1][:, H:][:, qs],
                                     wqkv[1][:, 2 * H:][:, qs]], 1)),
            wd0=wd[0][qs].copy(), wd1=wd[1][qs].copy(),
            cos2=bf(cos2), sin2=bf(sin2), rotT=bf(rot.T),
            onesr=np.ones((128, 128), np.float32),
            onesb=np.ones((128, 128), ml_dtypes.bfloat16),
            zeros=np.zeros((128, 512), np.float32),
            maskneg=bf(maskneg), resid=hTp[:, qs].copy(),
            encT=bf(f32(inputs["encoder_embeds"]).T),
            wk=bf(wkvf[:, :CC]), wvv=bf(wkvf[:, CC:]),
            wcq=(f32(inputs["w_cross_q"]) * wln_pa).copy(),
            wcd=f32(inputs["w_cross_dense"]),
            wgu0=bf(interleave(np.concatenate(
                [wgu[0][:, IS * c:IS * c + IS],
                 wgu[0][:, I + IS * c:I + IS * c + IS]], 1))),
            wgu1=bf(interleave(np.concatenate(
                [wgu[1][:, IS * c:IS * c + IS],
                 wgu[1][:, I + IS * c:I + IS * c + IS]], 1))),
            wdn0=bf(np.concatenate([wdn[0][IS * c:IS * c + IS],
                                    np.zeros((ISP - IS, H), np.float32)], 0)),
            wdn1=bf(np.concatenate([wdn[1][IS * c:IS * c + IS],
                                    np.zeros((ISP - IS, H), np.float32)], 0)),
        )
        in_maps.append(m)

    trace = bool(int(os.environ.get("KTRACE", "0")))
    res = run_bass_kernel_spmd(nc, in_maps, core_ids=list(range(NC_)),
                               trace=trace)
    kernel.last_exec_ns = res.exec_time_ns
    tot = res.results[0]["y"].astype(np.float64)
    for c in range(1, NC_):
        tot += res.results[c]["y"]
    for c in range(NC_):
        tot[:, 256 * c:256 * c + 256] += res.results[c]["h2out"]
    out = np.empty((S, H), np.float32)
    out[perm, :] = tot.T.astype(np.float32)
    return out


def kernel(**inputs):
    perm, b0, b1, b2 = _route(inputs)
    op = np.asarray(inputs["positions"])[perm].astype(np.int64)
    fast = (b0 % 256 == 0 and b1 % 256 == 0 and b2 % 256 == 0
            and op.size == S and np.all(np.diff(op) > 0))
    if fast and not bool(int(os.environ.get("KFORCE_GENERIC", "0"))):
        return _kernel_fast(inputs, perm, b0, b1, b2)
    return _kernel_generic(inputs, perm, b0, b1, b2)
